# revision 12
# baseline (speedup 1.0000x reference)
"""Bass/Tile TRN2 kernel for nn_AttentionHead: single-head attention with
q/k/v projections (512->64), key mask, softmax over 4096 keys.

Sharding: 8 cores; core c handles batch c//2, query-half c%2 (2048 queries),
with that batch's full k/v replicated. No collectives.

v4: host pre-casts to bf16, pre-transposes to [d, t], lays tiles out p-major
([128, block, chunk, t]) so every DMA is 128 contiguous 4KB descriptors, and
issues DMAs only on the two HWDGE rings (sync + scalar; gpsimd SWDGE was
serializing v3's prologue). All PE transposes are replaced by DMA XBAR
transposes: v and q rows are host-permuted within each 512-block by
PERM[j] = (j%4)*128 + j//4, which makes the [80,512] -> [128,4,80] XBAR
transpose land V's natural layout (and the q permutation cancels exactly
through the output transpose, so assembly is unchanged).

Per-core dataflow:
  - HWDGE loads qT/kT/vT bf16 [128, 4, 512] tiles (d on partitions)
  - QT/KT [128, t] = W^T x^T duplicated on partitions 64-127 via PE column
    packing (tile_position (0,0)/(0,64)) in one moving pass, evacuated by
    one DVE op (ACT engine is reserved exclusively for exp)
  - V: V^T proj psum -> one DVE scalar_tensor_tensor (+bias, *mask) ->
    [80, 512] staging (row 64 = mask row) -> XBAR transpose -> V1
    [t2, 80] whose col 64 is the masked-softmax denominator column
  - scores: S^T tiles [t2=128, 1024] = KT_chunk.T @ QT (contract e=64);
    chunk pairs run concurrently in array row groups 0-63/64-127
  - ScalarE: exp(0.125 * S^T), one call per [128, 1024] psum pair
  - PV: O^T[65, 512] += V1_chunk.T @ expS (row 64 = denominator); PV matmuls
    are emitted one tile behind the scores so the in-order PE never stalls
  - epilogue: psum -> bf16 [80, 512] -> XBAR transpose -> [128, 4, 80],
    reciprocal + scale on VectorE, p-major output DMA
  - query blocks 0,1 stream with the k/v blocks; 2,3 run after from
    SBUF-resident KT/V1 (PSUM bank budget: 4 scores + 2 PV + 2 proj)
"""

import sys
import types

import numpy as np
import ml_dtypes

import concourse.bass as bass
import concourse.tile as tile
from concourse import bacc, mybir
from concourse.masks import make_identity

B, T1, T2, D, E = 4, 4096, 4096, 512, 64
P = 128
T1L = T1 // 2          # queries per core
DC = D // P            # 4 d-chunks
NT2 = T2 // P          # 32 key chunks
TB = 512               # block size (t rows) for proj / k-v streaming
NBLK = T2 // TB        # 8 k/v blocks
NQB = T1L // TB        # 4 query blocks
CPQ = 2                # key chunks per score/exp tile
QW = CPQ * TB          # 1024
VR = 80                # padded row count for XBAR transposes (65 -> 80)
F32 = mybir.dt.float32
BF16 = mybir.dt.bfloat16
EXPF = mybir.ActivationFunctionType.Exp
ADD = mybir.AluOpType.add
MULT = mybir.AluOpType.mult

# within-512-block interleave so the XBAR transpose yields natural layout
PERM = (np.arange(TB) % 4) * P + np.arange(TB) // 4


def _install_ntff_hook():
    """Make trace=True usable under axon when antenv.axon_hooks is absent."""
    try:
        import antenv.axon_hooks  # noqa: F401
        return
    except ImportError:
        pass
    try:
        from trn_agent_boot.trn_boot import _ntff_profile_via_ctypes
        hook = _ntff_profile_via_ctypes("/opt/axon/libaxon_pjrt.so")
    except Exception:
        hook = None
    mod = types.ModuleType("antenv.axon_hooks")
    mod.get_axon_ntff_profile_hook = lambda: hook
    mod.set_axon_ntff_profile_hook = lambda h: None
    sys.modules["antenv.axon_hooks"] = mod


def build_body(tc, nc, qh, kh, vh, mrow, Wq, bq2, Wk, bk2, Wv, bv, out):
    with (
        tc.tile_pool(name="consts", bufs=1) as consts,
        tc.tile_pool(name="persist", bufs=1) as persist,
    ):
        bq_s = consts.tile([P, 1], F32)
        nc.scalar.dma_start(out=bq_s, in_=bq2[:, None])
        # warmup exp so the ACT table set loads during the prologue DMAs
        warm = consts.tile([P, 1], BF16)
        nc.scalar.activation(out=warm, in_=bq_s, func=EXPF, scale=1.0)

        wq_b = consts.tile([P, DC, E], BF16)
        nc.scalar.dma_start(out=wq_b, in_=Wq)
        wk_b = consts.tile([P, DC, E], BF16)
        nc.scalar.dma_start(out=wk_b, in_=Wk)
        wv_b = consts.tile([P, DC, E], BF16)
        nc.scalar.dma_start(out=wv_b, in_=Wv)

        bk_s = consts.tile([P, 1], F32)
        nc.scalar.dma_start(out=bk_s, in_=bk2[:, None])
        bv_s = consts.tile([E, 1], F32)
        nc.scalar.dma_start(out=bv_s, in_=bv[:, None])

        # per-key mask values in staged (block-permuted) order, bf16,
        # replicated across the 64 e-partitions (stride-0 partition DMA)
        mrow_s = consts.tile([E + 1, T2], BF16)
        nc.scalar.dma_start(
            out=mrow_s,
            in_=bass.AP(tensor=mrow.tensor, offset=mrow.offset,
                        ap=[[0, E + 1], mrow.ap[0]]))

        QT = persist.tile([P, T1L], BF16)
        KT = persist.tile([P, T2], BF16)
        V1 = persist.tile([P, NT2, VR], BF16)
        out_sb = persist.tile([P, T1L // P, E], F32)

        pv_tiles = {}
        pending = []

        with (
            tc.tile_pool(name="expp", bufs=4) as expp,
            tc.tile_pool(name="ep", bufs=3) as ep,
            tc.tile_pool(name="psS", bufs=2, space="PSUM") as psS,
            tc.tile_pool(name="psPV", bufs=1, space="PSUM") as psPV,
        ):
            def emit_pv(item):
                qb, qi, ex = item
                for u in range(CPQ):
                    c = CPQ * qi + u
                    nc.tensor.matmul(
                        pv_tiles[qb], V1[:, c, 0:E + 1],
                        ex[:, u * TB:(u + 1) * TB],
                        start=(c == 0), stop=(c == NT2 - 1))

            def scores_exp_pv(qb, qi):
                q0 = qb * TB
                ps = psS.tile([P, QW], F32, tag="s", name=f"s_{qb}_{qi}")
                for u in range(CPQ):
                    c = CPQ * qi + u
                    rg = E * (u % 2)
                    nc.tensor.matmul(
                        ps[:, u * TB:(u + 1) * TB],
                        KT[rg:rg + E, c * P:(c + 1) * P],
                        QT[rg:rg + E, q0:q0 + TB], start=True, stop=True,
                        tile_position=(rg, 0))
                ex = expp.tile([P, QW], BF16, tag="e", name=f"e_{qb}_{qi}")
                nc.scalar.activation(out=ex, in_=ps, func=EXPF, scale=0.125)
                pending.append((qb, qi, ex))
                while len(pending) > 1:
                    emit_pv(pending.pop(0))

            def flush_pv():
                while pending:
                    emit_pv(pending.pop(0))

            def epilogue(qb):
                pvt = pv_tiles.pop(qb)
                q0 = qb * TB
                ov = ep.tile([VR, TB], BF16, tag="ov", name=f"ov_{qb}")
                nc.gpsimd.memset(ov[E:VR, :], 0.0)
                nc.vector.tensor_copy(out=ov[0:E + 1, :], in_=pvt)
                po = ep.tile([P, TB // P, VR], BF16, tag="po",
                             name=f"po_{qb}")
                nc.sync.dma_start_transpose(out=po, in_=ov)
                for j in range(TB // P):
                    rec = ep.tile([P, 1], F32, tag="rec", name=f"rec_{qb}_{j}")
                    nc.vector.reciprocal(rec, po[:, j, E:E + 1])
                    nc.vector.tensor_scalar_mul(
                        out_sb[:, (q0 + j * P) // P, :], po[:, j, 0:E], rec)
                n0, n1 = q0 // P, (q0 + TB) // P
                nc.sync.dma_start(out=out[:, n0:n1, :],
                                  in_=out_sb[:, n0:n1, :])

            # ---------------- stage 1: stream ----------------
            stream_qbs = [0, 1]
            post_qbs = [2, 3]

            with (
                tc.tile_pool(name="qstage", bufs=NQB) as qstage,
                tc.tile_pool(name="kstage", bufs=NBLK) as kstage,
                tc.tile_pool(name="vstage", bufs=NBLK) as vstage,
                tc.tile_pool(name="psProj", bufs=2, space="PSUM") as psProj,
            ):
                for qb in stream_qbs:
                    pv_tiles[qb] = psPV.tile([E + 1, TB], F32,
                                             tag=f"pv{qb % 2}", name=f"pv_{qb}")

                # q on the scalar HWDGE ring (issued before any exp exists);
                # k/v batched on the sync ring, interleaved per block
                qst = {}
                for tb in range(NQB):
                    qst[tb] = qstage.tile([P, DC, TB], BF16, tag="st",
                                          name=f"st_q_{tb}")
                    nc.scalar.dma_start(out=qst[tb], in_=qh[:, tb])

                kst, vst = {}, {}

                def kv_dma(dst, pool, src, nm, b0, nb):
                    tl = pool.tile([P, nb, DC, TB], BF16, tag=f"st_{b0}",
                                   name=f"st_{nm}_{b0}", bufs=1)
                    nc.sync.dma_start(out=tl, in_=src[:, b0:b0 + nb])
                    for i in range(nb):
                        dst[b0 + i] = tl[:, i]

                kv_dma(kst, kstage, kh, "k", 0, 1)
                kv_dma(vst, vstage, vh, "v", 0, 1)
                for b0 in (1, 3, 5):
                    kv_dma(kst, kstage, kh, "k", b0, 2)
                    kv_dma(vst, vstage, vh, "v", b0, 2)
                kv_dma(kst, kstage, kh, "k", 7, 1)
                kv_dma(vst, vstage, vh, "v", 7, 1)

                def proj_dup(st, w_b, b_s, dst, tb):
                    # projection written to partitions 0:64 and duplicated to
                    # 64:128 in one moving pass via PE column packing
                    ps = psProj.tile([P, TB], F32, tag="proj",
                                     name=f"pp_{dst.tensor.name}_{tb}")
                    for j in range(DC):
                        nc.tensor.matmul(
                            ps[0:E, :], w_b[:, j], st[:, j],
                            start=(j == 0), stop=(j == DC - 1),
                            tile_position=(0, 0))
                        nc.tensor.matmul(
                            ps[E:P, :], w_b[:, j], st[:, j],
                            start=(j == 0), stop=(j == DC - 1),
                            tile_position=(0, E))
                    nc.vector.tensor_scalar_add(
                        dst[:, tb * TB:(tb + 1) * TB], ps, b_s)

                def vproj_block(tb):
                    st = vst.pop(tb)
                    ps = psProj.tile([E, TB], F32, tag="proj",
                                     name=f"psv_{tb}")
                    for j in range(DC):
                        nc.tensor.matmul(
                            ps, wv_b[:, j], st[:, j],
                            start=(j == 0), stop=(j == DC - 1))
                    bsl = slice(tb * TB, (tb + 1) * TB)
                    vm = ep.tile([VR, TB], BF16, tag="vm", name=f"vm_{tb}")
                    nc.gpsimd.memset(vm[E:VR, :], 0.0)
                    # fold bias and key mask into V rows in one DVE op:
                    # masked softmax = sum(mask*exp*V) / sum(mask*exp)
                    nc.vector.scalar_tensor_tensor(
                        out=vm[0:E, :], in0=ps, scalar=bv_s,
                        in1=mrow_s[0:E, bsl], op0=ADD, op1=MULT)
                    # row 64 carries the mask itself: the denominator column
                    nc.vector.tensor_copy(out=vm[E:E + 1, :],
                                          in_=mrow_s[E:E + 1, bsl])
                    nc.sync.dma_start_transpose(
                        out=V1[:, tb * (TB // P):(tb + 1) * (TB // P), :],
                        in_=vm)

                # block 0 first so exp starts as early as possible
                proj_dup(qst.pop(0), wq_b, bq_s, QT, 0)
                proj_dup(kst.pop(0), wk_b, bk_s, KT, 0)
                vproj_block(0)
                scores_exp_pv(0, 0)
                scores_exp_pv(0, 1)
                # remaining query-block projections
                for tb in range(1, NQB):
                    proj_dup(qst.pop(tb), wq_b, bq_s, QT, tb)
                    if tb == 1:
                        scores_exp_pv(1, 0)
                        scores_exp_pv(1, 1)
                for b in range(1, NBLK):
                    proj_dup(kst.pop(b), wk_b, bk_s, KT, b)
                    vproj_block(b)
                    for qb in stream_qbs:
                        for qi in range(b * CPQ, (b + 1) * CPQ):
                            scores_exp_pv(qb, qi)

            # ---------------- stage 2: remaining query blocks ----------------
            flush_pv()
            pending_stream_epi = list(stream_qbs)
            # free pv0 so the first post query block can start accumulating
            epilogue(pending_stream_epi.pop(0))

            def drain_stream_epi():
                while pending_stream_epi:
                    epilogue(pending_stream_epi.pop(0))

            for qb in post_qbs:
                pv_tiles[qb] = psPV.tile([E + 1, TB], F32,
                                         tag=f"pv{qb % 2}", name=f"pv_{qb}")
                for qi in range(NT2 // CPQ):
                    scores_exp_pv(qb, qi)
                    if qi >= 1:
                        drain_stream_epi()
                drain_stream_epi()
                flush_pv()
                epilogue(qb)


def build_nc(t1l=T1L, t2=T2):
    nc = bacc.Bacc()
    qh = nc.declare_dram_parameter("qh", [P, NQB, DC, TB], BF16, isOutput=False)
    kh = nc.declare_dram_parameter("kh", [P, NBLK, DC, TB], BF16,
                                   isOutput=False)
    vh = nc.declare_dram_parameter("vh", [P, NBLK, DC, TB], BF16,
                                   isOutput=False)
    mrow = nc.declare_dram_parameter("mrow", [T2], BF16, isOutput=False)
    Wq = nc.declare_dram_parameter("Wq", [P, DC, E], BF16, isOutput=False)
    bq2 = nc.declare_dram_parameter("bq2", [P], F32, isOutput=False)
    Wk = nc.declare_dram_parameter("Wk", [P, DC, E], BF16, isOutput=False)
    bk2 = nc.declare_dram_parameter("bk2", [P], F32, isOutput=False)
    Wv = nc.declare_dram_parameter("Wv", [P, DC, E], BF16, isOutput=False)
    bv = nc.declare_dram_parameter("bv", [E], F32, isOutput=False)
    out = nc.declare_dram_parameter("out", [P, T1L // P, E], F32,
                                    isOutput=True)
    with tile.TileContext(nc) as tc:
        build_body(tc, nc, qh[:], kh[:], vh[:], mrow[:], Wq[:], bq2[:],
                   Wk[:], bk2[:], Wv[:], bv[:], out[:])
    nc.compile()
    return nc


_NC_CACHE = {}


def _get_nc():
    if "nc" not in _NC_CACHE:
        _NC_CACHE["nc"] = build_nc()
    return _NC_CACHE["nc"]


def make_in_maps(q, k, v, mask, Wq, bq, Wk, bk, Wv, bv):
    bf = ml_dtypes.bfloat16
    f32 = np.float32

    def xh(x, nb):  # [t, d] fp32 -> [128, nb, 4, 512] bf16 p-major
        xt = np.asarray(x, f32).astype(bf).T             # [512, t]
        xt = xt.reshape(DC, P, nb, TB).transpose(1, 2, 0, 3)
        return np.ascontiguousarray(xt)

    def wh(W):  # [512, 64] fp32 -> [128, 4, 64] bf16 p-major
        Wr = np.asarray(W, f32).astype(bf).reshape(DC, P, E).transpose(1, 0, 2)
        return np.ascontiguousarray(Wr)

    shared = {
        "Wq": wh(Wq), "Wk": wh(Wk), "Wv": wh(Wv),
        "bq2": np.concatenate([bq, bq]).astype(f32),
        "bk2": np.concatenate([bk, bk]).astype(f32),
        "bv": np.ascontiguousarray(bv, f32),
    }
    per_b = []
    for b in range(B):
        per_b.append({
            "kh": xh(k[b], NBLK),
            "vh": xh(v[b], NBLK),
            "mrow": np.ascontiguousarray(
                np.asarray(mask[b, 0], f32).astype(bf)),
        })
    in_maps = []
    for c in range(8):
        b, h = divmod(c, 2)
        in_maps.append({
            "qh": xh(q[b, h * T1L:(h + 1) * T1L], NQB),
            **per_b[b],
            **shared,
        })
    return in_maps


def assemble_out(results):
    out = np.empty((B, T1, E), np.float32)
    for c in range(8):
        b, h = divmod(c, 2)
        # device out is [128, 16, 64] p-major -> [2048, 64]; the q block
        # permutation cancels exactly through the XBAR output transpose
        o = results[c]["out"].transpose(1, 0, 2).reshape(T1L, E)
        out[b, h * T1L:(h + 1) * T1L] = o
    return out


def run(inputs, trace=False):
    from concourse.bass_utils import run_bass_kernel_spmd
    _install_ntff_hook()
    nc = _get_nc()
    in_maps = make_in_maps(**inputs)
    res = run_bass_kernel_spmd(nc, in_maps, list(range(8)), trace=trace)
    return assemble_out(res.results), res


def kernel(q, k, v, mask, Wq, bq, Wk, bk, Wv, bv):
    out, _ = run(dict(q=q, k=k, v=v, mask=mask, Wq=Wq, bq=bq, Wk=Wk, bk=bk,
                      Wv=Wv, bv=bv))
    return out


# revision 13
# speedup vs baseline: 1.0573x; 1.0573x over previous
"""Bass/Tile TRN2 kernel for nn_AttentionHead: single-head attention with
q/k/v projections (512->64), key mask, softmax over 4096 keys.

Sharding: 8 cores; core c handles batch c//2, query-half c%2 (2048 queries),
with that batch's full k/v replicated. No collectives.

v5: host pre-casts to bf16, pre-transposes to [d, t], lays tiles out p-major
([128, block, chunk, t]) so every DMA is 128 contiguous 4KB descriptors.
DMAs are issued only on the two HWDGE rings: sync carries the first q/k/v
blocks then the batched k/v stream and outputs; scalar carries the small
constants (and q blocks 1-3) before its exp stream begins. The ACT warmup
(table load) is emitted after the scalar-ring DMA issues so it doesn't
serialize the prologue.

Per-core dataflow:
  - HWDGE loads qT/kT/vT bf16 [128, 4, 512] tiles (d on partitions)
  - QT/KT [128, t] = W^T x^T duplicated on partitions 64-127 via PE column
    packing (tile_position (0,0)/(0,64)) in one moving pass, evacuated by
    one DVE op (ACT engine is reserved exclusively for exp)
  - V: V^T proj psum -> one DVE scalar_tensor_tensor (+bias, *mask) ->
    bf16 [64, 512] -> PE transpose -> psum -> one DVE copy -> V1 [t2, 65]
    whose col 64 holds the mask (the masked-softmax denominator column:
    masked softmax == sum(mask*exp*V) / sum(mask*exp), no -1e9 bias)
  - scores: S^T tiles [t2=128, 1024] = KT_chunk.T @ QT (contract e=64);
    chunk pairs run concurrently in array row groups 0-63/64-127
  - ScalarE: exp(0.125 * S^T), one call per [128, 1024] psum pair
  - PV: O^T[65, 512] += V1_chunk.T @ expS (row 64 = denominator); PV matmuls
    are emitted one tile behind the scores so the in-order PE never stalls
  - epilogue: PE transpose [65,128] blocks, reciprocal + scale on VectorE,
    p-major output DMA
  - query blocks 0,1 stream with the k/v blocks; 2,3 run after from
    SBUF-resident KT/V1 (PSUM bank budget: 4 scores + 2 PV + 2 proj)
"""

import sys
import types

import numpy as np
import ml_dtypes

import concourse.bass as bass
import concourse.tile as tile
from concourse import bacc, mybir
from concourse.masks import make_identity

B, T1, T2, D, E = 4, 4096, 4096, 512, 64
P = 128
T1L = T1 // 2          # queries per core
DC = D // P            # 4 d-chunks
NT2 = T2 // P          # 32 key chunks
TB = 512               # block size (t rows) for proj / k-v streaming
NBLK = T2 // TB        # 8 k/v blocks
NQB = T1L // TB        # 4 query blocks
CPQ = 2                # key chunks per score/exp tile
QW = CPQ * TB          # 1024
F32 = mybir.dt.float32
BF16 = mybir.dt.bfloat16
EXPF = mybir.ActivationFunctionType.Exp
ADD = mybir.AluOpType.add
MULT = mybir.AluOpType.mult


def _install_ntff_hook():
    """Make trace=True usable under axon when antenv.axon_hooks is absent."""
    try:
        import antenv.axon_hooks  # noqa: F401
        return
    except ImportError:
        pass
    try:
        from trn_agent_boot.trn_boot import _ntff_profile_via_ctypes
        hook = _ntff_profile_via_ctypes("/opt/axon/libaxon_pjrt.so")
    except Exception:
        hook = None
    mod = types.ModuleType("antenv.axon_hooks")
    mod.get_axon_ntff_profile_hook = lambda: hook
    mod.set_axon_ntff_profile_hook = lambda h: None
    sys.modules["antenv.axon_hooks"] = mod


def build_body(tc, nc, qh, kh, vh, mh, mrow, Wq, bq2, Wk, bk2, Wv, bv, out):
    with (
        tc.tile_pool(name="consts", bufs=1) as consts,
        tc.tile_pool(name="persist", bufs=1) as persist,
        tc.tile_pool(name="qstage", bufs=NQB) as qstage,
        tc.tile_pool(name="kstage", bufs=1) as kstage,
        tc.tile_pool(name="vstage", bufs=1) as vstage,
    ):
        # ---- prologue DMAs: first blocks on sync, consts on scalar ----
        qst, kst, vst = {}, {}, {}

        def stage_q(tb, eng):
            qst[tb] = qstage.tile([P, DC, TB], BF16, tag="st",
                                  name=f"st_q_{tb}")
            eng.dma_start(out=qst[tb], in_=qh[:, tb])

        def kv_dma(dst, pool, src, nm, b0, nb):
            tl = pool.tile([P, nb, DC, TB], BF16, tag=f"st_{b0}",
                           name=f"st_{nm}_{b0}", bufs=1)
            nc.sync.dma_start(out=tl, in_=src[:, b0:b0 + nb])
            for i in range(nb):
                dst[b0 + i] = tl[:, i]

        stage_q(0, nc.sync)
        kv_dma(kst, kstage, kh, "k", 0, 1)
        kv_dma(vst, vstage, vh, "v", 0, 1)

        bq_s = consts.tile([P, 1], F32)
        nc.scalar.dma_start(out=bq_s, in_=bq2[:, None])
        wq_b = consts.tile([P, DC, E], BF16)
        nc.scalar.dma_start(out=wq_b, in_=Wq)
        wk_b = consts.tile([P, DC, E], BF16)
        nc.scalar.dma_start(out=wk_b, in_=Wk)
        wv_b = consts.tile([P, DC, E], BF16)
        nc.scalar.dma_start(out=wv_b, in_=Wv)
        bk_s = consts.tile([P, 1], F32)
        nc.scalar.dma_start(out=bk_s, in_=bk2[:, None])
        bv_s = consts.tile([E, 1], F32)
        nc.scalar.dma_start(out=bv_s, in_=bv[:, None])
        # per-key mask values, bf16, replicated across 65 partitions
        # (stride-0 partition DMA) for the V mask fold
        mrow_s = consts.tile([E + 1, T2], BF16)
        nc.scalar.dma_start(
            out=mrow_s,
            in_=bass.AP(tensor=mrow.tensor, offset=mrow.offset,
                        ap=[[0, E + 1], mrow.ap[0]]))
        # mask values per key, [partition = t2 % 128, col = t2 // 128]
        mk = consts.tile([P, NT2], F32)
        nc.scalar.dma_start(out=mk, in_=mh)

        for tb in range(1, NQB):
            stage_q(tb, nc.scalar)

        # remaining k/v stream, batched, on sync
        for b0 in (1, 3, 5):
            kv_dma(kst, kstage, kh, "k", b0, 2)
            kv_dma(vst, vstage, vh, "v", b0, 2)
        kv_dma(kst, kstage, kh, "k", 7, 1)
        kv_dma(vst, vstage, vh, "v", 7, 1)

        # warmup exp: loads the ACT table set now that the scalar ring is
        # done issuing prologue DMAs
        warm = consts.tile([P, 1], BF16)
        nc.scalar.activation(out=warm, in_=bq_s, func=EXPF, scale=1.0)

        ident_b = consts.tile([P, P], BF16)
        make_identity(nc, ident_b)
        ident_f = consts.tile([P, P], F32)
        make_identity(nc, ident_f)

        QT = persist.tile([P, T1L], BF16)
        KT = persist.tile([P, T2], BF16)
        V1 = persist.tile([P, NT2, E + 1], BF16)
        out_sb = persist.tile([P, T1L // P, E], F32)

        # the "ones" column of V1 carries the mask directly: the masked
        # softmax denominator is sum(mask * exp)
        nc.vector.tensor_copy(out=V1[:, :, E], in_=mk)

        pv_tiles = {}
        pending = []

        with (
            tc.tile_pool(name="expp", bufs=4) as expp,
            tc.tile_pool(name="ep", bufs=3) as ep,
            tc.tile_pool(name="psS", bufs=2, space="PSUM") as psS,
            tc.tile_pool(name="psPV", bufs=1, space="PSUM") as psPV,
        ):
            def emit_pv(item):
                qb, qi, ex = item
                for u in range(CPQ):
                    c = CPQ * qi + u
                    nc.tensor.matmul(
                        pv_tiles[qb], V1[:, c, :], ex[:, u * TB:(u + 1) * TB],
                        start=(c == 0), stop=(c == NT2 - 1))

            def scores_exp_pv(qb, qi):
                q0 = qb * TB
                ps = psS.tile([P, QW], F32, tag="s", name=f"s_{qb}_{qi}")
                for u in range(CPQ):
                    c = CPQ * qi + u
                    rg = E * (u % 2)
                    nc.tensor.matmul(
                        ps[:, u * TB:(u + 1) * TB],
                        KT[rg:rg + E, c * P:(c + 1) * P],
                        QT[rg:rg + E, q0:q0 + TB], start=True, stop=True,
                        tile_position=(rg, 0))
                ex = expp.tile([P, QW], BF16, tag="e", name=f"e_{qb}_{qi}")
                nc.scalar.activation(out=ex, in_=ps, func=EXPF, scale=0.125)
                pending.append((qb, qi, ex))
                while len(pending) > 1:
                    emit_pv(pending.pop(0))

            def flush_pv():
                while pending:
                    emit_pv(pending.pop(0))

            def epilogue(qb, psO):
                pvt = pv_tiles.pop(qb)
                q0 = qb * TB
                ov = ep.tile([E + 1, TB], F32, tag="ov", name=f"ov_{qb}")
                nc.vector.tensor_copy(out=ov, in_=pvt)
                for j in range(TB // P):
                    po = psO.tile([P, E + 1], F32, tag="o", name=f"o_{qb}_{j}")
                    nc.tensor.transpose(
                        po, ov[:, j * P:(j + 1) * P],
                        ident_f[0:E + 1, 0:E + 1])
                    rec = ep.tile([P, 1], F32, tag="rec", name=f"rec_{qb}_{j}")
                    nc.vector.reciprocal(rec, po[:, E:E + 1])
                    nc.vector.tensor_scalar_mul(
                        out_sb[:, (q0 + j * P) // P, :], po[:, 0:E], rec)
                n0, n1 = q0 // P, (q0 + TB) // P
                nc.sync.dma_start(out=out[:, n0:n1, :],
                                  in_=out_sb[:, n0:n1, :])

            # ---------------- stage 1: stream ----------------
            stream_qbs = [0, 1]
            post_qbs = [2, 3]

            with tc.tile_pool(name="psProj", bufs=2, space="PSUM") as psProj:
                for qb in stream_qbs:
                    pv_tiles[qb] = psPV.tile([E + 1, TB], F32,
                                             tag=f"pv{qb % 2}", name=f"pv_{qb}")

                def proj_dup(st, w_b, b_s, dst, tb):
                    # projection written to partitions 0:64 and duplicated to
                    # 64:128 in one moving pass via PE column packing
                    ps = psProj.tile([P, TB], F32, tag="proj",
                                     name=f"pp_{dst.tensor.name}_{tb}")
                    for j in range(DC):
                        nc.tensor.matmul(
                            ps[0:E, :], w_b[:, j], st[:, j],
                            start=(j == 0), stop=(j == DC - 1),
                            tile_position=(0, 0))
                        nc.tensor.matmul(
                            ps[E:P, :], w_b[:, j], st[:, j],
                            start=(j == 0), stop=(j == DC - 1),
                            tile_position=(0, E))
                    nc.vector.tensor_scalar_add(
                        dst[:, tb * TB:(tb + 1) * TB], ps, b_s)

                def vproj_block(tb):
                    st = vst.pop(tb)
                    ps = psProj.tile([E, TB], F32, tag="proj",
                                     name=f"psv_{tb}")
                    for j in range(DC):
                        nc.tensor.matmul(
                            ps, wv_b[:, j], st[:, j],
                            start=(j == 0), stop=(j == DC - 1))
                    bsl = slice(tb * TB, (tb + 1) * TB)
                    vm = ep.tile([E, TB], BF16, tag="vm", name=f"vm_{tb}")
                    # fold bias and key mask into V rows in one DVE op:
                    # masked softmax = sum(mask*exp*V) / sum(mask*exp)
                    nc.vector.scalar_tensor_tensor(
                        out=vm, in0=ps, scalar=bv_s,
                        in1=mrow_s[0:E, bsl], op0=ADD, op1=MULT)
                    pvt = psProj.tile([P, TB // P, E], BF16, tag="proj",
                                      name=f"vt_{tb}")
                    for ci in range(TB // P):
                        nc.tensor.transpose(
                            pvt[:, ci], vm[:, ci * P:(ci + 1) * P],
                            ident_b[0:E, 0:E])
                    nc.vector.tensor_copy(
                        out=V1[:, tb * (TB // P):(tb + 1) * (TB // P), 0:E],
                        in_=pvt)

                # block 0 first so exp starts as early as possible
                proj_dup(qst.pop(0), wq_b, bq_s, QT, 0)
                proj_dup(kst.pop(0), wk_b, bk_s, KT, 0)
                vproj_block(0)
                scores_exp_pv(0, 0)
                scores_exp_pv(0, 1)
                # remaining query-block projections
                for tb in range(1, NQB):
                    proj_dup(qst.pop(tb), wq_b, bq_s, QT, tb)
                    if tb == 1:
                        scores_exp_pv(1, 0)
                        scores_exp_pv(1, 1)
                for b in range(1, NBLK):
                    proj_dup(kst.pop(b), wk_b, bk_s, KT, b)
                    vproj_block(b)
                    for qb in stream_qbs:
                        for qi in range(b * CPQ, (b + 1) * CPQ):
                            scores_exp_pv(qb, qi)

            # ---------------- stage 2: remaining query blocks ----------------
            with tc.tile_pool(name="psO", bufs=2, space="PSUM") as psO:
                flush_pv()
                pending_stream_epi = list(stream_qbs)
                # free pv0 so the first post query block can start accumulating
                epilogue(pending_stream_epi.pop(0), psO)

                def drain_stream_epi():
                    while pending_stream_epi:
                        epilogue(pending_stream_epi.pop(0), psO)

                for qb in post_qbs:
                    pv_tiles[qb] = psPV.tile([E + 1, TB], F32,
                                             tag=f"pv{qb % 2}", name=f"pv_{qb}")
                    for qi in range(NT2 // CPQ):
                        scores_exp_pv(qb, qi)
                        if qi >= 1:
                            drain_stream_epi()
                    drain_stream_epi()
                    flush_pv()
                    epilogue(qb, psO)


def build_nc(t1l=T1L, t2=T2):
    nc = bacc.Bacc()
    qh = nc.declare_dram_parameter("qh", [P, NQB, DC, TB], BF16, isOutput=False)
    kh = nc.declare_dram_parameter("kh", [P, NBLK, DC, TB], BF16,
                                   isOutput=False)
    vh = nc.declare_dram_parameter("vh", [P, NBLK, DC, TB], BF16,
                                   isOutput=False)
    mh = nc.declare_dram_parameter("mh", [P, NT2], F32, isOutput=False)
    mrow = nc.declare_dram_parameter("mrow", [T2], BF16, isOutput=False)
    Wq = nc.declare_dram_parameter("Wq", [P, DC, E], BF16, isOutput=False)
    bq2 = nc.declare_dram_parameter("bq2", [P], F32, isOutput=False)
    Wk = nc.declare_dram_parameter("Wk", [P, DC, E], BF16, isOutput=False)
    bk2 = nc.declare_dram_parameter("bk2", [P], F32, isOutput=False)
    Wv = nc.declare_dram_parameter("Wv", [P, DC, E], BF16, isOutput=False)
    bv = nc.declare_dram_parameter("bv", [E], F32, isOutput=False)
    out = nc.declare_dram_parameter("out", [P, T1L // P, E], F32,
                                    isOutput=True)
    with tile.TileContext(nc) as tc:
        build_body(tc, nc, qh[:], kh[:], vh[:], mh[:], mrow[:], Wq[:], bq2[:],
                   Wk[:], bk2[:], Wv[:], bv[:], out[:])
    nc.compile()
    return nc


_NC_CACHE = {}


def _get_nc():
    if "nc" not in _NC_CACHE:
        _NC_CACHE["nc"] = build_nc()
    return _NC_CACHE["nc"]


def make_in_maps(q, k, v, mask, Wq, bq, Wk, bk, Wv, bv):
    bf = ml_dtypes.bfloat16
    f32 = np.float32

    def xh(x, nb):  # [t, d] fp32 -> [128, nb, 4, 512] bf16 p-major
        xt = np.asarray(x, f32).astype(bf).T             # [512, t]
        xt = xt.reshape(DC, P, nb, TB).transpose(1, 2, 0, 3)
        return np.ascontiguousarray(xt)

    def wh(W):  # [512, 64] fp32 -> [128, 4, 64] bf16 p-major
        Wr = np.asarray(W, f32).astype(bf).reshape(DC, P, E).transpose(1, 0, 2)
        return np.ascontiguousarray(Wr)

    shared = {
        "Wq": wh(Wq), "Wk": wh(Wk), "Wv": wh(Wv),
        "bq2": np.concatenate([bq, bq]).astype(f32),
        "bk2": np.concatenate([bk, bk]).astype(f32),
        "bv": np.ascontiguousarray(bv, f32),
    }
    per_b = []
    for b in range(B):
        m = np.asarray(mask[b, 0], f32)
        per_b.append({
            "kh": xh(k[b], NBLK),
            "vh": xh(v[b], NBLK),
            "mh": np.ascontiguousarray(m.reshape(NT2, P).T),
            "mrow": np.ascontiguousarray(m.astype(bf)),
        })
    in_maps = []
    for c in range(8):
        b, h = divmod(c, 2)
        in_maps.append({
            "qh": xh(q[b, h * T1L:(h + 1) * T1L], NQB),
            **per_b[b],
            **shared,
        })
    return in_maps


def assemble_out(results):
    out = np.empty((B, T1, E), np.float32)
    for c in range(8):
        b, h = divmod(c, 2)
        # device out is [128, 16, 64] p-major -> [2048, 64]
        o = results[c]["out"].transpose(1, 0, 2).reshape(T1L, E)
        out[b, h * T1L:(h + 1) * T1L] = o
    return out


def run(inputs, trace=False):
    from concourse.bass_utils import run_bass_kernel_spmd
    _install_ntff_hook()
    nc = _get_nc()
    in_maps = make_in_maps(**inputs)
    res = run_bass_kernel_spmd(nc, in_maps, list(range(8)), trace=trace)
    return assemble_out(res.results), res


def kernel(q, k, v, mask, Wq, bq, Wk, bk, Wv, bv):
    out, _ = run(dict(q=q, k=k, v=v, mask=mask, Wq=Wq, bq=bq, Wk=Wk, bk=bk,
                      Wv=Wv, bv=bv))
    return out


# revision 14
# speedup vs baseline: 1.1423x; 1.0804x over previous
"""Bass/Tile TRN2 kernel for nn_AttentionHead: single-head attention with
q/k/v projections (512->64), key mask, softmax over 4096 keys.

Sharding: 8 cores; core c handles batch c//2, query-half c%2 (2048 queries),
with that batch's full k/v replicated. No collectives.

v6: on top of v5 (host bf16 cast + [d,t] transpose + p-major layouts, all
DMA on the two HWDGE rings, column-packed dup projections, DVE mask fold,
PE kept exclusively for matmul work and ACT for exp):
  - keys are sorted per batch so masked keys come last, and a fixed count
    of trailing masked keys is dropped (they contribute exactly zero to
    both the softmax numerator and denominator, so this is exact). The
    kernel compiles a 29-chunk (3712-key) variant used when every batch
    has >= 384 masked keys, and falls back to the full 32-chunk variant
    otherwise.
  - k/v stream DMAs issued per block in consumption order (early blocks
    individually, later ones batched) so the PE never waits on a batch
    that also carries a later block.

Per-core dataflow:
  - HWDGE loads qT/kT/vT bf16 [128, nch, 4, 128] tiles (d on partitions)
  - QT/KT [128, t] = W^T x^T duplicated on partitions 64-127 via PE column
    packing (tile_position (0,0)/(0,64)) in one moving pass, one DVE evac
  - V: V^T proj psum -> one DVE scalar_tensor_tensor (+bias, *mask) ->
    bf16 -> PE transpose -> psum -> one DVE copy -> V1 [t2, 65] whose
    col 64 holds the mask (masked softmax == sum(mask*exp*V)/sum(mask*exp))
  - scores: S^T tiles [t2=128, 1024] = KT_chunk.T @ QT (contract e=64);
    chunk pairs run concurrently in array row groups 0-63/64-127
  - ScalarE: exp(0.125 * S^T), one call per [128, 1024] psum pair
  - PV: O^T[65, 512] += V1_chunk.T @ expS (row 64 = denominator); PV
    matmuls are emitted one tile behind the scores so the PE never stalls
  - epilogue: PE transpose [65,128] blocks, reciprocal + scale on VectorE
  - query blocks 0,1 stream with the k/v blocks; 2,3 run after from
    SBUF-resident KT/V1 (PSUM bank budget: 4 scores + 2 PV + 2 proj)
"""

import sys
import types

import numpy as np
import ml_dtypes

import concourse.bass as bass
import concourse.tile as tile
from concourse import bacc, mybir
from concourse.masks import make_identity

B, T1, T2, D, E = 4, 4096, 4096, 512, 64
P = 128
T1L = T1 // 2          # queries per core
DC = D // P            # 4 d-chunks
TB = 512               # q block size / nominal k-v block size
NQB = T1L // TB        # 4 query blocks
NT2_FULL = T2 // P     # 32 key chunks
NT2_CUT = 29           # 3712 keys: used when every batch has >=384 masked
F32 = mybir.dt.float32
BF16 = mybir.dt.bfloat16
EXPF = mybir.ActivationFunctionType.Exp
ADD = mybir.AluOpType.add
MULT = mybir.AluOpType.mult


def _install_ntff_hook():
    """Make trace=True usable under axon when antenv.axon_hooks is absent."""
    try:
        import antenv.axon_hooks  # noqa: F401
        return
    except ImportError:
        pass
    try:
        from trn_agent_boot.trn_boot import _ntff_profile_via_ctypes
        hook = _ntff_profile_via_ctypes("/opt/axon/libaxon_pjrt.so")
    except Exception:
        hook = None
    mod = types.ModuleType("antenv.axon_hooks")
    mod.get_axon_ntff_profile_hook = lambda: hook
    mod.set_axon_ntff_profile_hook = lambda h: None
    sys.modules["antenv.axon_hooks"] = mod


def _blocks(nt2):
    """Key chunk blocks [(c0, nch)], 4-chunk blocks with a ragged tail."""
    return [(c0, min(4, nt2 - c0)) for c0 in range(0, nt2, 4)]


def _pairs(c0, nch):
    """Score-tile chunk groups within a block (pairs, ragged tail)."""
    return [(c, min(2, c0 + nch - c)) for c in range(c0, c0 + nch, 2)]


def build_body(tc, nc, nt2, qh, kh, vh, mh, mrow, Wq, bq2, Wk, bk2, Wv, bv,
               out):
    t2 = nt2 * P
    blocks = _blocks(nt2)
    with (
        tc.tile_pool(name="consts", bufs=1) as consts,
        tc.tile_pool(name="persist", bufs=1) as persist,
        tc.tile_pool(name="qstage", bufs=NQB) as qstage,
        tc.tile_pool(name="kstage", bufs=1) as kstage,
        tc.tile_pool(name="vstage", bufs=1) as vstage,
    ):
        # ---- prologue DMAs: first blocks on sync, consts on scalar ----
        qst, kst, vst = {}, {}, {}

        def stage_q(tb, eng):
            qst[tb] = qstage.tile([P, DC, TB], BF16, tag="st",
                                  name=f"st_q_{tb}")
            eng.dma_start(out=qst[tb], in_=qh[:, tb])

        def kv_dma(dst, pool, src, nm, bis):
            c0 = blocks[bis[0]][0]
            nch = sum(blocks[bi][1] for bi in bis)
            tl = pool.tile([P, nch, DC, P], BF16, tag=f"st_{c0}",
                           name=f"st_{nm}_{c0}", bufs=1)
            nc.sync.dma_start(out=tl, in_=src[:, c0:c0 + nch])
            for bi in bis:
                b0, bn = blocks[bi]
                dst[bi] = tl[:, b0 - c0:b0 - c0 + bn]

        stage_q(0, nc.sync)
        kv_dma(kst, kstage, kh, "k", [0])
        kv_dma(vst, vstage, vh, "v", [0])
        kv_dma(kst, kstage, kh, "k", [1])
        kv_dma(vst, vstage, vh, "v", [1])

        bq_s = consts.tile([P, 1], F32)
        nc.scalar.dma_start(out=bq_s, in_=bq2[:, None])
        wq_b = consts.tile([P, DC, E], BF16)
        nc.scalar.dma_start(out=wq_b, in_=Wq)
        wk_b = consts.tile([P, DC, E], BF16)
        nc.scalar.dma_start(out=wk_b, in_=Wk)
        wv_b = consts.tile([P, DC, E], BF16)
        nc.scalar.dma_start(out=wv_b, in_=Wv)
        bk_s = consts.tile([P, 1], F32)
        nc.scalar.dma_start(out=bk_s, in_=bk2[:, None])
        bv_s = consts.tile([E, 1], F32)
        nc.scalar.dma_start(out=bv_s, in_=bv[:, None])
        for tb in range(1, NQB):
            stage_q(tb, nc.scalar)
        # per-key mask values, bf16, replicated across 64 partitions
        # (stride-0 partition DMA) for the V mask fold
        mrow_s = consts.tile([E, t2], BF16)
        nc.scalar.dma_start(
            out=mrow_s,
            in_=bass.AP(tensor=mrow.tensor, offset=mrow.offset,
                        ap=[[0, E], mrow.ap[0]]))
        # mask values per key, [partition = t2 % 128, col = t2 // 128]
        mk = consts.tile([P, nt2], F32)
        nc.scalar.dma_start(out=mk, in_=mh)

        # remaining k/v stream, batched, on sync
        nb = len(blocks)
        groups = [[2, 3], [4, 5], [6, 7]]
        for g in groups:
            g = [bi for bi in g if bi < nb]
            if g:
                kv_dma(kst, kstage, kh, "k", g)
                kv_dma(vst, vstage, vh, "v", g)

        # warmup exp: loads the ACT table set now that the scalar ring is
        # done issuing prologue DMAs
        warm = consts.tile([P, 1], BF16)
        nc.scalar.activation(out=warm, in_=bq_s, func=EXPF, scale=1.0)

        ident_b = consts.tile([P, P], BF16)
        make_identity(nc, ident_b)
        ident_f = consts.tile([P, P], F32)
        make_identity(nc, ident_f)

        QT = persist.tile([P, T1L], BF16)
        KT = persist.tile([P, t2], BF16)
        V1 = persist.tile([P, nt2, E + 1], BF16)
        out_sb = persist.tile([P, T1L // P, E], F32)

        # the "ones" column of V1 carries the mask directly: the masked
        # softmax denominator is sum(mask * exp)
        nc.vector.tensor_copy(out=V1[:, :, E], in_=mk)

        pv_tiles = {}
        pending = []

        with (
            tc.tile_pool(name="expp", bufs=4) as expp,
            tc.tile_pool(name="ep", bufs=3) as ep,
            tc.tile_pool(name="psS", bufs=2, space="PSUM") as psS,
            tc.tile_pool(name="psPV", bufs=1, space="PSUM") as psPV,
        ):
            def emit_pv(item):
                qb, c0, ncp, ex = item
                for u in range(ncp):
                    c = c0 + u
                    nc.tensor.matmul(
                        pv_tiles[qb], V1[:, c, :], ex[:, u * TB:(u + 1) * TB],
                        start=(c == 0), stop=(c == nt2 - 1))

            def scores_exp_pv(qb, c0, ncp):
                q0 = qb * TB
                w = ncp * TB
                ps = psS.tile([P, w], F32, tag="s", name=f"s_{qb}_{c0}")
                for u in range(ncp):
                    c = c0 + u
                    rg = E * (u % 2)
                    nc.tensor.matmul(
                        ps[:, u * TB:(u + 1) * TB],
                        KT[rg:rg + E, c * P:(c + 1) * P],
                        QT[rg:rg + E, q0:q0 + TB], start=True, stop=True,
                        tile_position=(rg, 0))
                ex = expp.tile([P, w], BF16, tag="e", name=f"e_{qb}_{c0}")
                nc.scalar.activation(out=ex, in_=ps, func=EXPF, scale=0.125)
                pending.append((qb, c0, ncp, ex))
                while len(pending) > 1:
                    emit_pv(pending.pop(0))

            def flush_pv():
                while pending:
                    emit_pv(pending.pop(0))

            def epilogue(qb, psO):
                pvt = pv_tiles.pop(qb)
                q0 = qb * TB
                ov = ep.tile([E + 1, TB], F32, tag="ov", name=f"ov_{qb}")
                nc.vector.tensor_copy(out=ov, in_=pvt)
                for j in range(TB // P):
                    po = psO.tile([P, E + 1], F32, tag="o", name=f"o_{qb}_{j}")
                    nc.tensor.transpose(
                        po, ov[:, j * P:(j + 1) * P],
                        ident_f[0:E + 1, 0:E + 1])
                    rec = ep.tile([P, 1], F32, tag="rec", name=f"rec_{qb}_{j}")
                    nc.vector.reciprocal(rec, po[:, E:E + 1])
                    nc.vector.tensor_scalar_mul(
                        out_sb[:, (q0 + j * P) // P, :], po[:, 0:E], rec)
                n0, n1 = q0 // P, (q0 + TB) // P
                nc.sync.dma_start(out=out[:, n0:n1, :],
                                  in_=out_sb[:, n0:n1, :])

            # ---------------- stage 1: stream ----------------
            stream_qbs = [0, 1]
            post_qbs = [2, 3]

            with tc.tile_pool(name="psProj", bufs=2, space="PSUM") as psProj:
                for qb in stream_qbs:
                    pv_tiles[qb] = psPV.tile([E + 1, TB], F32,
                                             tag=f"pv{qb % 2}", name=f"pv_{qb}")

                def proj_dup(st, w_b, b_s, dst, c0, nch):
                    # projection written to partitions 0:64 and duplicated to
                    # 64:128 in one moving pass via PE column packing
                    w = nch * P
                    ps = psProj.tile([P, w], F32, tag="proj",
                                     name=f"pp_{dst.tensor.name}_{c0}")
                    for j in range(DC):
                        nc.tensor.matmul(
                            ps[0:E, :], w_b[:, j], st[:, :, j, :],
                            start=(j == 0), stop=(j == DC - 1),
                            tile_position=(0, 0))
                        nc.tensor.matmul(
                            ps[E:P, :], w_b[:, j], st[:, :, j, :],
                            start=(j == 0), stop=(j == DC - 1),
                            tile_position=(0, E))
                    nc.vector.tensor_scalar_add(
                        dst[:, c0 * P:c0 * P + w], ps, b_s)

                def qproj(st, tb):
                    # q staging keeps the [P, DC, TB] layout
                    ps = psProj.tile([P, TB], F32, tag="proj",
                                     name=f"pp_q_{tb}")
                    for j in range(DC):
                        nc.tensor.matmul(
                            ps[0:E, :], wq_b[:, j], st[:, j],
                            start=(j == 0), stop=(j == DC - 1),
                            tile_position=(0, 0))
                        nc.tensor.matmul(
                            ps[E:P, :], wq_b[:, j], st[:, j],
                            start=(j == 0), stop=(j == DC - 1),
                            tile_position=(0, E))
                    nc.vector.tensor_scalar_add(
                        QT[:, tb * TB:(tb + 1) * TB], ps, bq_s)

                def vproj_block(bi):
                    c0, nch = blocks[bi]
                    st = vst.pop(bi)
                    w = nch * P
                    ps = psProj.tile([E, w], F32, tag="proj",
                                     name=f"psv_{c0}")
                    for j in range(DC):
                        nc.tensor.matmul(
                            ps, wv_b[:, j], st[:, :, j, :],
                            start=(j == 0), stop=(j == DC - 1))
                    bsl = slice(c0 * P, c0 * P + w)
                    vm = ep.tile([E, w], BF16, tag="vm", name=f"vm_{c0}")
                    # fold bias and key mask into V rows in one DVE op:
                    # masked softmax = sum(mask*exp*V) / sum(mask*exp)
                    nc.vector.scalar_tensor_tensor(
                        out=vm, in0=ps, scalar=bv_s,
                        in1=mrow_s[:, bsl], op0=ADD, op1=MULT)
                    pvt = psProj.tile([P, nch, E], BF16, tag="proj",
                                      name=f"vt_{c0}")
                    for ci in range(nch):
                        nc.tensor.transpose(
                            pvt[:, ci], vm[:, ci * P:(ci + 1) * P],
                            ident_b[0:E, 0:E])
                    nc.vector.tensor_copy(out=V1[:, c0:c0 + nch, 0:E],
                                          in_=pvt)

                def block_scores(bi, qbs):
                    c0, nch = blocks[bi]
                    for qb in qbs:
                        for cp, ncp in _pairs(c0, nch):
                            scores_exp_pv(qb, cp, ncp)

                # block 0 first so exp starts as early as possible
                qproj(qst.pop(0), 0)
                proj_dup(kst.pop(0), wk_b, bk_s, KT, *blocks[0])
                vproj_block(0)
                block_scores(0, [0])
                # remaining query-block projections
                for tb in range(1, NQB):
                    qproj(qst.pop(tb), tb)
                    if tb == 1:
                        block_scores(0, [1])
                for bi in range(1, len(blocks)):
                    proj_dup(kst.pop(bi), wk_b, bk_s, KT, *blocks[bi])
                    vproj_block(bi)
                    block_scores(bi, stream_qbs)

            # ---------------- stage 2: remaining query blocks ----------------
            with tc.tile_pool(name="psO", bufs=2, space="PSUM") as psO:
                flush_pv()
                pending_stream_epi = list(stream_qbs)
                # free pv0 so the first post query block can start accumulating
                epilogue(pending_stream_epi.pop(0), psO)

                def drain_stream_epi():
                    while pending_stream_epi:
                        epilogue(pending_stream_epi.pop(0), psO)

                for qb in post_qbs:
                    pv_tiles[qb] = psPV.tile([E + 1, TB], F32,
                                             tag=f"pv{qb % 2}", name=f"pv_{qb}")
                    first = True
                    for bi in range(len(blocks)):
                        block_scores(bi, [qb])
                        if not first:
                            drain_stream_epi()
                        first = False
                    drain_stream_epi()
                    flush_pv()
                    epilogue(qb, psO)


def build_nc(nt2):
    t2 = nt2 * P
    nc = bacc.Bacc()
    qh = nc.declare_dram_parameter("qh", [P, NQB, DC, TB], BF16, isOutput=False)
    kh = nc.declare_dram_parameter("kh", [P, nt2, DC, P], BF16, isOutput=False)
    vh = nc.declare_dram_parameter("vh", [P, nt2, DC, P], BF16, isOutput=False)
    mh = nc.declare_dram_parameter("mh", [P, nt2], F32, isOutput=False)
    mrow = nc.declare_dram_parameter("mrow", [t2], BF16, isOutput=False)
    Wq = nc.declare_dram_parameter("Wq", [P, DC, E], BF16, isOutput=False)
    bq2 = nc.declare_dram_parameter("bq2", [P], F32, isOutput=False)
    Wk = nc.declare_dram_parameter("Wk", [P, DC, E], BF16, isOutput=False)
    bk2 = nc.declare_dram_parameter("bk2", [P], F32, isOutput=False)
    Wv = nc.declare_dram_parameter("Wv", [P, DC, E], BF16, isOutput=False)
    bv = nc.declare_dram_parameter("bv", [E], F32, isOutput=False)
    out = nc.declare_dram_parameter("out", [P, T1L // P, E], F32,
                                    isOutput=True)
    with tile.TileContext(nc) as tc:
        build_body(tc, nc, nt2, qh[:], kh[:], vh[:], mh[:], mrow[:], Wq[:],
                   bq2[:], Wk[:], bk2[:], Wv[:], bv[:], out[:])
    nc.compile()
    return nc


_NC_CACHE = {}


def _get_nc(nt2):
    if nt2 not in _NC_CACHE:
        _NC_CACHE[nt2] = build_nc(nt2)
    return _NC_CACHE[nt2]


def make_in_maps(nt2, q, k, v, mask, Wq, bq, Wk, bk, Wv, bv):
    bf = ml_dtypes.bfloat16
    f32 = np.float32
    t2k = nt2 * P

    def qx(x):  # [t, 512] fp32 -> [128, NQB, 4, 512] bf16 p-major
        xt = np.asarray(x, f32).astype(bf).T
        xt = xt.reshape(DC, P, NQB, TB).transpose(1, 2, 0, 3)
        return np.ascontiguousarray(xt)

    def kx(x):  # [t2k, 512] fp32 -> [128, nt2, 4, 128] bf16 p-major
        xt = np.asarray(x, f32).astype(bf).T              # [512, t2k]
        xt = xt.reshape(DC, P, nt2, P).transpose(1, 2, 0, 3)
        return np.ascontiguousarray(xt)

    def wh(W):
        Wr = np.asarray(W, f32).astype(bf).reshape(DC, P, E).transpose(1, 0, 2)
        return np.ascontiguousarray(Wr)

    shared = {
        "Wq": wh(Wq), "Wk": wh(Wk), "Wv": wh(Wv),
        "bq2": np.concatenate([bq, bq]).astype(f32),
        "bk2": np.concatenate([bk, bk]).astype(f32),
        "bv": np.ascontiguousarray(bv, f32),
    }
    per_b = []
    for b in range(B):
        m = np.asarray(mask[b, 0], f32)
        # masked keys last; dropping trailing masked keys is exact
        order = np.argsort(1.0 - m, kind="stable")[:t2k]
        ms = m[order]
        per_b.append({
            "kh": kx(np.asarray(k[b], f32)[order]),
            "vh": kx(np.asarray(v[b], f32)[order]),
            "mh": np.ascontiguousarray(ms.reshape(nt2, P).T),
            "mrow": np.ascontiguousarray(ms.astype(bf)),
        })
    in_maps = []
    for c in range(8):
        b, h = divmod(c, 2)
        in_maps.append({
            "qh": qx(q[b, h * T1L:(h + 1) * T1L]),
            **per_b[b],
            **shared,
        })
    return in_maps


def assemble_out(results):
    out = np.empty((B, T1, E), np.float32)
    for c in range(8):
        b, h = divmod(c, 2)
        # device out is [128, 16, 64] p-major -> [2048, 64]
        o = results[c]["out"].transpose(1, 0, 2).reshape(T1L, E)
        out[b, h * T1L:(h + 1) * T1L] = o
    return out


def run(inputs, trace=False):
    from concourse.bass_utils import run_bass_kernel_spmd
    _install_ntff_hook()
    # the cut variant is exact when every batch keeps all unmasked keys
    unmasked = np.asarray(inputs["mask"], np.float32).reshape(B, T2)
    max_keep = int((unmasked != 0.0).sum(axis=1).max())
    nt2 = NT2_CUT if max_keep <= NT2_CUT * P else NT2_FULL
    nc = _get_nc(nt2)
    in_maps = make_in_maps(nt2, **inputs)
    res = run_bass_kernel_spmd(nc, in_maps, list(range(8)), trace=trace)
    return assemble_out(res.results), res


def kernel(q, k, v, mask, Wq, bq, Wk, bk, Wv, bv):
    out, _ = run(dict(q=q, k=k, v=v, mask=mask, Wq=Wq, bq=bq, Wk=Wk, bk=bk,
                      Wv=Wv, bv=bv))
    return out


# revision 16
# speedup vs baseline: 1.2351x; 1.0812x over previous
"""Bass/Tile TRN2 kernel for nn_AttentionHead: single-head attention with
q/k/v projections (512->64), key mask, softmax over 4096 keys.

Sharding: 8 cores; core c handles batch c//2, query-half c%2 (2048 queries),
with that batch's full k/v replicated. No collectives.

v6: on top of v5 (host bf16 cast + [d,t] transpose + p-major layouts, all
DMA on the two HWDGE rings, column-packed dup projections, DVE mask fold,
PE kept exclusively for matmul work and ACT for exp):
  - keys are sorted per batch so masked keys come last, and a fixed count
    of trailing masked keys is dropped (they contribute exactly zero to
    both the softmax numerator and denominator, so this is exact). The
    kernel compiles a 29-chunk (3712-key) variant used when every batch
    has >= 384 masked keys, and falls back to the full 32-chunk variant
    otherwise.
  - k/v stream DMAs issued per block in consumption order (early blocks
    individually, later ones batched) so the PE never waits on a batch
    that also carries a later block.

Per-core dataflow:
  - HWDGE loads qT/kT/vT bf16 [128, nch, 4, 128] tiles (d on partitions)
  - QT/KT [128, t] = W^T x^T duplicated on partitions 64-127 via PE column
    packing (tile_position (0,0)/(0,64)) in one moving pass, one DVE evac
  - V: V^T proj psum -> one DVE scalar_tensor_tensor (+bias, *mask) ->
    bf16 -> PE transpose -> psum -> one DVE copy -> V1 [t2, 65] whose
    col 64 holds the mask (masked softmax == sum(mask*exp*V)/sum(mask*exp))
  - scores: S^T tiles [t2=128, 1024] = KT_chunk.T @ QT (contract e=64);
    chunk pairs run concurrently in array row groups 0-63/64-127
  - ScalarE: exp(0.125 * S^T), one call per [128, 1024] psum pair
  - PV: O^T[65, 512] += V1_chunk.T @ expS (row 64 = denominator); PV
    matmuls are emitted one tile behind the scores so the PE never stalls
  - epilogue: PE transpose [65,128] blocks, reciprocal + scale on VectorE
  - query blocks 0,1 stream with the k/v blocks; 2,3 run after from
    SBUF-resident KT/V1 (PSUM bank budget: 4 scores + 2 PV + 2 proj)
"""

import sys
import types

import numpy as np
import ml_dtypes

import concourse.bass as bass
import concourse.tile as tile
from concourse import bacc, mybir
from concourse.masks import make_identity

B, T1, T2, D, E = 4, 4096, 4096, 512, 64
P = 128
T1L = T1 // 2          # queries per core
DC = D // P            # 4 d-chunks
TB = 512               # q block size / nominal k-v block size
NQB = T1L // TB        # 4 query blocks
NT2_FULL = T2 // P     # 32 key chunks
NT2_CUT = 29           # 3712 keys: used when every batch has >=384 masked
F32 = mybir.dt.float32
BF16 = mybir.dt.bfloat16
EXPF = mybir.ActivationFunctionType.Exp
ADD = mybir.AluOpType.add
MULT = mybir.AluOpType.mult


def _install_ntff_hook():
    """Make trace=True usable under axon when antenv.axon_hooks is absent."""
    try:
        import antenv.axon_hooks  # noqa: F401
        return
    except ImportError:
        pass
    try:
        from trn_agent_boot.trn_boot import _ntff_profile_via_ctypes
        hook = _ntff_profile_via_ctypes("/opt/axon/libaxon_pjrt.so")
    except Exception:
        hook = None
    mod = types.ModuleType("antenv.axon_hooks")
    mod.get_axon_ntff_profile_hook = lambda: hook
    mod.set_axon_ntff_profile_hook = lambda h: None
    sys.modules["antenv.axon_hooks"] = mod


def _blocks(nt2):
    """Key chunk blocks [(c0, nch)], 4-chunk blocks with a ragged tail."""
    return [(c0, min(4, nt2 - c0)) for c0 in range(0, nt2, 4)]


def _pairs(c0, nch):
    """Score-tile chunk groups within a block (pairs, ragged tail)."""
    return [(c, min(2, c0 + nch - c)) for c in range(c0, c0 + nch, 2)]


def build_body(tc, nc, nt2, qh, kh, vh, mh, mrow, Wq, bq2, Wk, bk2, Wv, bv,
               out):
    t2 = nt2 * P
    blocks = _blocks(nt2)
    with (
        tc.tile_pool(name="consts", bufs=1) as consts,
        tc.tile_pool(name="persist", bufs=1) as persist,
        tc.tile_pool(name="qstage", bufs=NQB) as qstage,
        tc.tile_pool(name="kstage", bufs=1) as kstage,
        tc.tile_pool(name="vstage", bufs=1) as vstage,
    ):
        # ---- prologue DMAs: first blocks on sync, consts on scalar ----
        qst, kst, vst = {}, {}, {}

        def stage_q(tb, eng):
            qst[tb] = qstage.tile([P, DC, TB], BF16, tag="st",
                                  name=f"st_q_{tb}")
            eng.dma_start(out=qst[tb], in_=qh[:, tb])

        def kv_dma(dst, pool, src, nm, bi):
            # tag-rotated staging (bufs=3) self-throttles the k/v stream to
            # consumption pace: later blocks transfer just-in-time instead
            # of fighting the early blocks for DMA/SBUF bandwidth
            c0, nch = blocks[bi]
            tl = pool.tile([P, nch, DC, P], BF16, tag="st",
                           name=f"st_{nm}_{c0}", bufs=3)
            nc.sync.dma_start(out=tl, in_=src[:, c0:c0 + nch])
            dst[bi] = tl

        stage_q(0, nc.sync)
        kv_dma(kst, kstage, kh, "k", 0)
        kv_dma(vst, vstage, vh, "v", 0)
        kv_dma(kst, kstage, kh, "k", 1)
        kv_dma(vst, vstage, vh, "v", 1)

        bq_s = consts.tile([P, 1], F32)
        nc.scalar.dma_start(out=bq_s, in_=bq2[:, None])
        wq_b = consts.tile([P, DC, E], BF16)
        nc.scalar.dma_start(out=wq_b, in_=Wq)
        wk_b = consts.tile([P, DC, E], BF16)
        nc.scalar.dma_start(out=wk_b, in_=Wk)
        wv_b = consts.tile([P, DC, E], BF16)
        nc.scalar.dma_start(out=wv_b, in_=Wv)
        bk_s = consts.tile([P, 1], F32)
        nc.scalar.dma_start(out=bk_s, in_=bk2[:, None])
        bv_s = consts.tile([E, 1], F32)
        nc.scalar.dma_start(out=bv_s, in_=bv[:, None])
        stage_q(1, nc.scalar)
        # per-key mask values, bf16, replicated across 64 partitions
        # (stride-0 partition DMA) for the V mask fold
        mrow_s = consts.tile([E, t2], BF16)
        nc.scalar.dma_start(
            out=mrow_s,
            in_=bass.AP(tensor=mrow.tensor, offset=mrow.offset,
                        ap=[[0, E], mrow.ap[0]]))
        # mask values per key, [partition = t2 % 128, col = t2 // 128]
        mk = consts.tile([P, nt2], F32)
        nc.scalar.dma_start(out=mk, in_=mh)

        # remaining k/v stream on sync, throttled by staging rotation;
        # q blocks 2-3 (needed only in stage 2) ride between early blocks
        nb = len(blocks)
        for bi in range(2, nb):
            kv_dma(kst, kstage, kh, "k", bi)
            kv_dma(vst, vstage, vh, "v", bi)
            if bi == 2:
                stage_q(2, nc.sync)
                stage_q(3, nc.sync)

        # warmup exp: loads the ACT table set now that the scalar ring is
        # done issuing prologue DMAs
        warm = consts.tile([P, 1], BF16)
        nc.scalar.activation(out=warm, in_=bq_s, func=EXPF, scale=1.0)

        ident_b = consts.tile([P, P], BF16)
        make_identity(nc, ident_b)
        ident_f = consts.tile([P, P], F32)
        make_identity(nc, ident_f)

        QT = persist.tile([P, T1L], BF16)
        KT = persist.tile([P, t2], BF16)
        V1 = persist.tile([P, nt2, E + 1], BF16)
        out_sb = persist.tile([P, T1L // P, E], F32)

        # the "ones" column of V1 carries the mask directly: the masked
        # softmax denominator is sum(mask * exp)
        nc.vector.tensor_copy(out=V1[:, :, E], in_=mk)

        pv_tiles = {}
        pending = []

        with (
            tc.tile_pool(name="expp", bufs=4) as expp,
            tc.tile_pool(name="ep", bufs=3) as ep,
            tc.tile_pool(name="psS", bufs=2, space="PSUM") as psS,
            tc.tile_pool(name="psPV", bufs=1, space="PSUM") as psPV,
        ):
            def emit_pv(item):
                qb, c0, ncp, ex = item
                for u in range(ncp):
                    c = c0 + u
                    nc.tensor.matmul(
                        pv_tiles[qb], V1[:, c, :], ex[:, u * TB:(u + 1) * TB],
                        start=(c == 0), stop=(c == nt2 - 1))

            def scores_exp_pv(qb, c0, ncp):
                q0 = qb * TB
                w = ncp * TB
                ps = psS.tile([P, w], F32, tag="s", name=f"s_{qb}_{c0}")
                for u in range(ncp):
                    c = c0 + u
                    rg = E * (u % 2)
                    nc.tensor.matmul(
                        ps[:, u * TB:(u + 1) * TB],
                        KT[rg:rg + E, c * P:(c + 1) * P],
                        QT[rg:rg + E, q0:q0 + TB], start=True, stop=True,
                        tile_position=(rg, 0))
                ex = expp.tile([P, w], BF16, tag="e", name=f"e_{qb}_{c0}")
                nc.scalar.activation(out=ex, in_=ps, func=EXPF, scale=0.125)
                pending.append((qb, c0, ncp, ex))
                while len(pending) > 1:
                    emit_pv(pending.pop(0))

            def flush_pv():
                while pending:
                    emit_pv(pending.pop(0))

            def epilogue(qb, psO):
                pvt = pv_tiles.pop(qb)
                q0 = qb * TB
                ov = ep.tile([E + 1, TB], F32, tag="ov", name=f"ov_{qb}")
                nc.vector.tensor_copy(out=ov, in_=pvt)
                for j in range(TB // P):
                    po = psO.tile([P, E + 1], F32, tag="o", name=f"o_{qb}_{j}")
                    nc.tensor.transpose(
                        po, ov[:, j * P:(j + 1) * P],
                        ident_f[0:E + 1, 0:E + 1])
                    rec = ep.tile([P, 1], F32, tag="rec", name=f"rec_{qb}_{j}")
                    nc.vector.reciprocal(rec, po[:, E:E + 1])
                    nc.vector.tensor_scalar_mul(
                        out_sb[:, (q0 + j * P) // P, :], po[:, 0:E], rec)
                n0, n1 = q0 // P, (q0 + TB) // P
                nc.sync.dma_start(out=out[:, n0:n1, :],
                                  in_=out_sb[:, n0:n1, :])

            # ---------------- stage 1: stream ----------------
            stream_qbs = [0, 1]
            post_qbs = [2, 3]

            with tc.tile_pool(name="psProj", bufs=2, space="PSUM") as psProj:
                for qb in stream_qbs:
                    pv_tiles[qb] = psPV.tile([E + 1, TB], F32,
                                             tag=f"pv{qb % 2}", name=f"pv_{qb}")

                def proj_dup(st, w_b, b_s, dst, c0, nch):
                    # projection written to partitions 0:64 and duplicated to
                    # 64:128 in one moving pass via PE column packing
                    w = nch * P
                    ps = psProj.tile([P, w], F32, tag="proj",
                                     name=f"pp_{dst.tensor.name}_{c0}")
                    for j in range(DC):
                        nc.tensor.matmul(
                            ps[0:E, :], w_b[:, j], st[:, :, j, :],
                            start=(j == 0), stop=(j == DC - 1),
                            tile_position=(0, 0))
                        nc.tensor.matmul(
                            ps[E:P, :], w_b[:, j], st[:, :, j, :],
                            start=(j == 0), stop=(j == DC - 1),
                            tile_position=(0, E))
                    nc.vector.tensor_scalar_add(
                        dst[:, c0 * P:c0 * P + w], ps, b_s)

                def qproj(st, tb):
                    # q staging keeps the [P, DC, TB] layout
                    ps = psProj.tile([P, TB], F32, tag="proj",
                                     name=f"pp_q_{tb}")
                    for j in range(DC):
                        nc.tensor.matmul(
                            ps[0:E, :], wq_b[:, j], st[:, j],
                            start=(j == 0), stop=(j == DC - 1),
                            tile_position=(0, 0))
                        nc.tensor.matmul(
                            ps[E:P, :], wq_b[:, j], st[:, j],
                            start=(j == 0), stop=(j == DC - 1),
                            tile_position=(0, E))
                    nc.vector.tensor_scalar_add(
                        QT[:, tb * TB:(tb + 1) * TB], ps, bq_s)

                def vproj_block(bi):
                    c0, nch = blocks[bi]
                    st = vst.pop(bi)
                    w = nch * P
                    ps = psProj.tile([E, w], F32, tag="proj",
                                     name=f"psv_{c0}")
                    for j in range(DC):
                        nc.tensor.matmul(
                            ps, wv_b[:, j], st[:, :, j, :],
                            start=(j == 0), stop=(j == DC - 1))
                    bsl = slice(c0 * P, c0 * P + w)
                    vm = ep.tile([E, w], BF16, tag="vm", name=f"vm_{c0}")
                    # fold bias and key mask into V rows in one DVE op:
                    # masked softmax = sum(mask*exp*V) / sum(mask*exp)
                    nc.vector.scalar_tensor_tensor(
                        out=vm, in0=ps, scalar=bv_s,
                        in1=mrow_s[:, bsl], op0=ADD, op1=MULT)
                    pvt = psProj.tile([P, nch, E], BF16, tag="proj",
                                      name=f"vt_{c0}")
                    for ci in range(nch):
                        nc.tensor.transpose(
                            pvt[:, ci], vm[:, ci * P:(ci + 1) * P],
                            ident_b[0:E, 0:E])
                    nc.vector.tensor_copy(out=V1[:, c0:c0 + nch, 0:E],
                                          in_=pvt)

                def block_scores(bi, qbs):
                    c0, nch = blocks[bi]
                    for qb in qbs:
                        for cp, ncp in _pairs(c0, nch):
                            scores_exp_pv(qb, cp, ncp)

                # block 0 first so exp starts as early as possible; q blocks
                # 2/3 project later (their staging DMAs arrive mid-stream)
                qproj(qst.pop(0), 0)
                proj_dup(kst.pop(0), wk_b, bk_s, KT, *blocks[0])
                p0 = _pairs(*blocks[0])
                scores_exp_pv(0, *p0[0])
                vproj_block(0)
                for cp, ncp in p0[1:]:
                    scores_exp_pv(0, cp, ncp)
                qproj(qst.pop(1), 1)
                block_scores(0, [1])
                for bi in range(1, len(blocks)):
                    proj_dup(kst.pop(bi), wk_b, bk_s, KT, *blocks[bi])
                    vproj_block(bi)
                    block_scores(bi, stream_qbs)
                    if bi == 1:
                        qproj(qst.pop(2), 2)
                    elif bi == 2:
                        qproj(qst.pop(3), 3)

            # ---------------- stage 2: remaining query blocks ----------------
            with tc.tile_pool(name="psO", bufs=2, space="PSUM") as psO:
                flush_pv()
                pending_stream_epi = list(stream_qbs)
                # free pv0 so the first post query block can start accumulating
                epilogue(pending_stream_epi.pop(0), psO)

                def drain_stream_epi():
                    while pending_stream_epi:
                        epilogue(pending_stream_epi.pop(0), psO)

                for qb in post_qbs:
                    pv_tiles[qb] = psPV.tile([E + 1, TB], F32,
                                             tag=f"pv{qb % 2}", name=f"pv_{qb}")
                    first = True
                    for bi in range(len(blocks)):
                        block_scores(bi, [qb])
                        if not first:
                            drain_stream_epi()
                        first = False
                    drain_stream_epi()
                    flush_pv()
                    epilogue(qb, psO)


def build_nc(nt2):
    t2 = nt2 * P
    nc = bacc.Bacc()
    qh = nc.declare_dram_parameter("qh", [P, NQB, DC, TB], BF16, isOutput=False)
    kh = nc.declare_dram_parameter("kh", [P, nt2, DC, P], BF16, isOutput=False)
    vh = nc.declare_dram_parameter("vh", [P, nt2, DC, P], BF16, isOutput=False)
    mh = nc.declare_dram_parameter("mh", [P, nt2], F32, isOutput=False)
    mrow = nc.declare_dram_parameter("mrow", [t2], BF16, isOutput=False)
    Wq = nc.declare_dram_parameter("Wq", [P, DC, E], BF16, isOutput=False)
    bq2 = nc.declare_dram_parameter("bq2", [P], F32, isOutput=False)
    Wk = nc.declare_dram_parameter("Wk", [P, DC, E], BF16, isOutput=False)
    bk2 = nc.declare_dram_parameter("bk2", [P], F32, isOutput=False)
    Wv = nc.declare_dram_parameter("Wv", [P, DC, E], BF16, isOutput=False)
    bv = nc.declare_dram_parameter("bv", [E], F32, isOutput=False)
    out = nc.declare_dram_parameter("out", [P, T1L // P, E], F32,
                                    isOutput=True)
    with tile.TileContext(nc) as tc:
        build_body(tc, nc, nt2, qh[:], kh[:], vh[:], mh[:], mrow[:], Wq[:],
                   bq2[:], Wk[:], bk2[:], Wv[:], bv[:], out[:])
    nc.compile()
    return nc


_NC_CACHE = {}


def _get_nc(nt2):
    if nt2 not in _NC_CACHE:
        _NC_CACHE[nt2] = build_nc(nt2)
    return _NC_CACHE[nt2]


def make_in_maps(nt2, q, k, v, mask, Wq, bq, Wk, bk, Wv, bv):
    bf = ml_dtypes.bfloat16
    f32 = np.float32
    t2k = nt2 * P

    def qx(x):  # [t, 512] fp32 -> [128, NQB, 4, 512] bf16 p-major
        xt = np.asarray(x, f32).astype(bf).T
        xt = xt.reshape(DC, P, NQB, TB).transpose(1, 2, 0, 3)
        return np.ascontiguousarray(xt)

    def kx(x):  # [t2k, 512] fp32 -> [128, nt2, 4, 128] bf16 p-major
        xt = np.asarray(x, f32).astype(bf).T              # [512, t2k]
        xt = xt.reshape(DC, P, nt2, P).transpose(1, 2, 0, 3)
        return np.ascontiguousarray(xt)

    def wh(W):
        Wr = np.asarray(W, f32).astype(bf).reshape(DC, P, E).transpose(1, 0, 2)
        return np.ascontiguousarray(Wr)

    shared = {
        "Wq": wh(Wq), "Wk": wh(Wk), "Wv": wh(Wv),
        "bq2": np.concatenate([bq, bq]).astype(f32),
        "bk2": np.concatenate([bk, bk]).astype(f32),
        "bv": np.ascontiguousarray(bv, f32),
    }
    per_b = []
    for b in range(B):
        m = np.asarray(mask[b, 0], f32)
        # masked keys last; dropping trailing masked keys is exact
        order = np.argsort(1.0 - m, kind="stable")[:t2k]
        ms = m[order]
        per_b.append({
            "kh": kx(np.asarray(k[b], f32)[order]),
            "vh": kx(np.asarray(v[b], f32)[order]),
            "mh": np.ascontiguousarray(ms.reshape(nt2, P).T),
            "mrow": np.ascontiguousarray(ms.astype(bf)),
        })
    in_maps = []
    for c in range(8):
        b, h = divmod(c, 2)
        in_maps.append({
            "qh": qx(q[b, h * T1L:(h + 1) * T1L]),
            **per_b[b],
            **shared,
        })
    return in_maps


def assemble_out(results):
    out = np.empty((B, T1, E), np.float32)
    for c in range(8):
        b, h = divmod(c, 2)
        # device out is [128, 16, 64] p-major -> [2048, 64]
        o = results[c]["out"].transpose(1, 0, 2).reshape(T1L, E)
        out[b, h * T1L:(h + 1) * T1L] = o
    return out


def run(inputs, trace=False):
    from concourse.bass_utils import run_bass_kernel_spmd
    _install_ntff_hook()
    # the cut variant is exact when every batch keeps all unmasked keys
    unmasked = np.asarray(inputs["mask"], np.float32).reshape(B, T2)
    max_keep = int((unmasked != 0.0).sum(axis=1).max())
    nt2 = NT2_CUT if max_keep <= NT2_CUT * P else NT2_FULL
    nc = _get_nc(nt2)
    in_maps = make_in_maps(nt2, **inputs)
    res = run_bass_kernel_spmd(nc, in_maps, list(range(8)), trace=trace)
    return assemble_out(res.results), res


def kernel(q, k, v, mask, Wq, bq, Wk, bk, Wv, bv):
    out, _ = run(dict(q=q, k=k, v=v, mask=mask, Wq=Wq, bq=bq, Wk=Wk, bk=bk,
                      Wv=Wv, bv=bv))
    return out


# revision 20
# speedup vs baseline: 1.2365x; 1.0011x over previous
"""Bass/Tile TRN2 kernel for nn_AttentionHead: single-head attention with
q/k/v projections (512->64), key mask, softmax over 4096 keys.

Sharding: 8 cores; core c handles batch c//2, query-half c%2 (2048 queries),
with that batch's full k/v replicated. No collectives.

v6: on top of v5 (host bf16 cast + [d,t] transpose + p-major layouts, all
DMA on the two HWDGE rings, column-packed dup projections, DVE mask fold,
PE kept exclusively for matmul work and ACT for exp):
  - keys are sorted per batch so masked keys come last, and a fixed count
    of trailing masked keys is dropped (they contribute exactly zero to
    both the softmax numerator and denominator, so this is exact). The
    kernel compiles a 29-chunk (3712-key) variant used when every batch
    has >= 384 masked keys, and falls back to the full 32-chunk variant
    otherwise.
  - k/v stream DMAs issued per block in consumption order (early blocks
    individually, later ones batched) so the PE never waits on a batch
    that also carries a later block.

Per-core dataflow:
  - HWDGE loads qT/kT/vT bf16 [128, nch, 4, 128] tiles (d on partitions)
  - QT/KT [128, t] = W^T x^T duplicated on partitions 64-127 via PE column
    packing (tile_position (0,0)/(0,64)) in one moving pass, one DVE evac
  - V: V^T proj psum -> one DVE scalar_tensor_tensor (+bias, *mask) ->
    bf16 -> PE transpose -> psum -> one DVE copy -> V1 [t2, 65] whose
    col 64 holds the mask (masked softmax == sum(mask*exp*V)/sum(mask*exp))
  - scores: S^T tiles [t2=128, 1024] = KT_chunk.T @ QT (contract e=64);
    chunk pairs run concurrently in array row groups 0-63/64-127
  - ScalarE: exp(0.125 * S^T), one call per [128, 1024] psum pair
  - PV: O^T[65, 512] += V1_chunk.T @ expS (row 64 = denominator); PV
    matmuls are emitted one tile behind the scores so the PE never stalls
  - epilogue: PE transpose [65,128] blocks, reciprocal + scale on VectorE
  - query blocks 0,1 stream with the k/v blocks; 2,3 run after from
    SBUF-resident KT/V1 (PSUM bank budget: 4 scores + 2 PV + 2 proj)
"""

import sys
import types

import numpy as np
import ml_dtypes

import concourse.bass as bass
import concourse.tile as tile
from concourse import bacc, mybir
from concourse.masks import make_identity

B, T1, T2, D, E = 4, 4096, 4096, 512, 64
P = 128
T1L = T1 // 2          # queries per core
DC = D // P            # 4 d-chunks
TB = 512               # q block size / nominal k-v block size
NQB = T1L // TB        # 4 query blocks
NT2_FULL = T2 // P     # 32 key chunks
NT2_CUT = 29           # 3712 keys: used when every batch has >=384 masked
F32 = mybir.dt.float32
BF16 = mybir.dt.bfloat16
EXPF = mybir.ActivationFunctionType.Exp
ADD = mybir.AluOpType.add
MULT = mybir.AluOpType.mult


def _install_ntff_hook():
    """Make trace=True usable under axon when antenv.axon_hooks is absent."""
    try:
        import antenv.axon_hooks  # noqa: F401
        return
    except ImportError:
        pass
    try:
        from trn_agent_boot.trn_boot import _ntff_profile_via_ctypes
        hook = _ntff_profile_via_ctypes("/opt/axon/libaxon_pjrt.so")
    except Exception:
        hook = None
    mod = types.ModuleType("antenv.axon_hooks")
    mod.get_axon_ntff_profile_hook = lambda: hook
    mod.set_axon_ntff_profile_hook = lambda h: None
    sys.modules["antenv.axon_hooks"] = mod


def _blocks(nt2):
    """Key chunk blocks [(c0, nch)], 4-chunk blocks with a ragged tail."""
    return [(c0, min(4, nt2 - c0)) for c0 in range(0, nt2, 4)]


def _pairs(c0, nch):
    """Score-tile chunk groups within a block (pairs, ragged tail)."""
    return [(c, min(2, c0 + nch - c)) for c in range(c0, c0 + nch, 2)]


def build_body(tc, nc, nt2, qh, kh, vh, mh, mrow, Wq, bq2, Wk, bk2, Wv, bv,
               out):
    t2 = nt2 * P
    blocks = _blocks(nt2)
    with (
        tc.tile_pool(name="consts", bufs=1) as consts,
        tc.tile_pool(name="persist", bufs=1) as persist,
        tc.tile_pool(name="qstage", bufs=NQB) as qstage,
        tc.tile_pool(name="kstage", bufs=1) as kstage,
        tc.tile_pool(name="vstage", bufs=1) as vstage,
    ):
        # ---- prologue DMAs: first blocks on sync, consts on scalar ----
        qst, kst, vst = {}, {}, {}

        def stage_q(tb, eng):
            qst[tb] = qstage.tile([P, DC, TB], BF16, tag="st",
                                  name=f"st_q_{tb}")
            eng.dma_start(out=qst[tb], in_=qh[:, tb])

        def kv_dma(dst, pool, src, nm, bi):
            # tag-rotated staging (bufs=3) self-throttles the k/v stream to
            # consumption pace: later blocks transfer just-in-time instead
            # of fighting the early blocks for DMA/SBUF bandwidth
            c0, nch = blocks[bi]
            tl = pool.tile([P, nch, DC, P], BF16, tag="st",
                           name=f"st_{nm}_{c0}", bufs=3)
            nc.sync.dma_start(out=tl, in_=src[:, c0:c0 + nch])
            dst[bi] = tl

        stage_q(0, nc.sync)
        kv_dma(kst, kstage, kh, "k", 0)
        kv_dma(kst, kstage, kh, "k", 1)
        kv_dma(vst, vstage, vh, "v", 0)
        kv_dma(vst, vstage, vh, "v", 1)

        bq_s = consts.tile([P, 1], F32)
        nc.scalar.dma_start(out=bq_s, in_=bq2[:, None])
        wq_b = consts.tile([P, DC, E], BF16)
        nc.scalar.dma_start(out=wq_b, in_=Wq)
        wk_b = consts.tile([P, DC, E], BF16)
        nc.scalar.dma_start(out=wk_b, in_=Wk)
        wv_b = consts.tile([P, DC, E], BF16)
        nc.scalar.dma_start(out=wv_b, in_=Wv)
        bk_s = consts.tile([P, 1], F32)
        nc.scalar.dma_start(out=bk_s, in_=bk2[:, None])
        bv_s = consts.tile([E, 1], F32)
        nc.scalar.dma_start(out=bv_s, in_=bv[:, None])
        stage_q(1, nc.scalar)
        # per-key mask values, bf16, replicated across 64 partitions
        # (stride-0 partition DMA) for the V mask fold
        mrow_s = consts.tile([E, t2], BF16)
        nc.scalar.dma_start(
            out=mrow_s,
            in_=bass.AP(tensor=mrow.tensor, offset=mrow.offset,
                        ap=[[0, E], mrow.ap[0]]))
        # mask values per key, [partition = t2 % 128, col = t2 // 128]
        mk = consts.tile([P, nt2], F32)
        nc.scalar.dma_start(out=mk, in_=mh)

        # remaining k/v stream on sync, throttled by staging rotation;
        # q blocks 2-3 (needed only in stage 2) ride between early blocks
        nb = len(blocks)
        for bi in range(2, nb):
            kv_dma(kst, kstage, kh, "k", bi)
            kv_dma(vst, vstage, vh, "v", bi)
            if bi == 2:
                stage_q(2, nc.sync)
                stage_q(3, nc.sync)

        # warmup exp: loads the ACT table set now that the scalar ring is
        # done issuing prologue DMAs
        warm = consts.tile([P, 1], BF16)
        nc.scalar.activation(out=warm, in_=bq_s, func=EXPF, scale=1.0)

        ident_b = consts.tile([P, P], BF16)
        make_identity(nc, ident_b)
        ident_f = consts.tile([P, P], F32)
        make_identity(nc, ident_f)

        QT = persist.tile([P, T1L], BF16)
        KT = persist.tile([P, t2], BF16)
        V1 = persist.tile([P, nt2, E + 1], BF16)
        out_sb = persist.tile([P, T1L // P, E], F32)

        # the "ones" column of V1 carries the mask directly: the masked
        # softmax denominator is sum(mask * exp)
        nc.vector.tensor_copy(out=V1[:, :, E], in_=mk)

        pv_tiles = {}
        pending = []

        with (
            tc.tile_pool(name="expp", bufs=4) as expp,
            tc.tile_pool(name="ep", bufs=3) as ep,
            tc.tile_pool(name="psS", bufs=2, space="PSUM") as psS,
            tc.tile_pool(name="psPV", bufs=1, space="PSUM") as psPV,
        ):
            def emit_pv(item):
                qb, c0, ncp, ex = item
                for u in range(ncp):
                    c = c0 + u
                    nc.tensor.matmul(
                        pv_tiles[qb], V1[:, c, :], ex[:, u * TB:(u + 1) * TB],
                        start=(c == 0), stop=(c == nt2 - 1))

            hold_pv = [False]

            def scores_exp_pv(qb, c0, ncp):
                q0 = qb * TB
                w = ncp * TB
                ps = psS.tile([P, w], F32, tag="s", name=f"s_{qb}_{c0}")
                for u in range(ncp):
                    c = c0 + u
                    rg = E * (u % 2)
                    nc.tensor.matmul(
                        ps[:, u * TB:(u + 1) * TB],
                        KT[rg:rg + E, c * P:(c + 1) * P],
                        QT[rg:rg + E, q0:q0 + TB], start=True, stop=True,
                        tile_position=(rg, 0))
                ex = expp.tile([P, w], BF16, tag="e", name=f"e_{qb}_{c0}")
                nc.scalar.activation(out=ex, in_=ps, func=EXPF, scale=0.125)
                pending.append((qb, c0, ncp, ex))
                while not hold_pv[0] and len(pending) > 1:
                    emit_pv(pending.pop(0))

            def flush_pv():
                while pending:
                    emit_pv(pending.pop(0))

            def epilogue(qb, psO):
                pvt = pv_tiles.pop(qb)
                q0 = qb * TB
                ov = ep.tile([E + 1, TB], F32, tag="ov", name=f"ov_{qb}")
                nc.vector.tensor_copy(out=ov, in_=pvt)
                for j in range(TB // P):
                    po = psO.tile([P, E + 1], F32, tag="o", name=f"o_{qb}_{j}")
                    nc.tensor.transpose(
                        po, ov[:, j * P:(j + 1) * P],
                        ident_f[0:E + 1, 0:E + 1])
                    rec = ep.tile([P, 1], F32, tag="rec", name=f"rec_{qb}_{j}")
                    nc.vector.reciprocal(rec, po[:, E:E + 1])
                    n = (q0 + j * P) // P
                    nc.vector.tensor_scalar_mul(
                        out_sb[:, n, :], po[:, 0:E], rec)
                    # per-chunk output DMA keeps the final-epilogue tail short
                    nc.sync.dma_start(out=out[:, n:n + 1, :],
                                      in_=out_sb[:, n:n + 1, :])

            # ---------------- stage 1: stream ----------------
            stream_qbs = [0, 1]
            post_qbs = [2, 3]

            with tc.tile_pool(name="psProj", bufs=2, space="PSUM") as psProj:
                for qb in stream_qbs:
                    pv_tiles[qb] = psPV.tile([E + 1, TB], F32,
                                             tag=f"pv{qb % 2}", name=f"pv_{qb}")

                def proj_dup(st, w_b, b_s, dst, c0, nch):
                    # projection written to partitions 0:64 and duplicated to
                    # 64:128 in one moving pass via PE column packing
                    w = nch * P
                    ps = psProj.tile([P, w], F32, tag="proj",
                                     name=f"pp_{dst.tensor.name}_{c0}")
                    for j in range(DC):
                        nc.tensor.matmul(
                            ps[0:E, :], w_b[:, j], st[:, :, j, :],
                            start=(j == 0), stop=(j == DC - 1),
                            tile_position=(0, 0))
                        nc.tensor.matmul(
                            ps[E:P, :], w_b[:, j], st[:, :, j, :],
                            start=(j == 0), stop=(j == DC - 1),
                            tile_position=(0, E))
                    nc.vector.tensor_scalar_add(
                        dst[:, c0 * P:c0 * P + w], ps, b_s)

                def qproj(st, tb):
                    # q staging keeps the [P, DC, TB] layout
                    ps = psProj.tile([P, TB], F32, tag="proj",
                                     name=f"pp_q_{tb}")
                    for j in range(DC):
                        nc.tensor.matmul(
                            ps[0:E, :], wq_b[:, j], st[:, j],
                            start=(j == 0), stop=(j == DC - 1),
                            tile_position=(0, 0))
                        nc.tensor.matmul(
                            ps[E:P, :], wq_b[:, j], st[:, j],
                            start=(j == 0), stop=(j == DC - 1),
                            tile_position=(0, E))
                    nc.vector.tensor_scalar_add(
                        QT[:, tb * TB:(tb + 1) * TB], ps, bq_s)

                def vproj_block(bi):
                    c0, nch = blocks[bi]
                    st = vst.pop(bi)
                    w = nch * P
                    ps = psProj.tile([E, w], F32, tag="proj",
                                     name=f"psv_{c0}")
                    for j in range(DC):
                        nc.tensor.matmul(
                            ps, wv_b[:, j], st[:, :, j, :],
                            start=(j == 0), stop=(j == DC - 1))
                    bsl = slice(c0 * P, c0 * P + w)
                    vm = ep.tile([E, w], BF16, tag="vm", name=f"vm_{c0}")
                    # fold bias and key mask into V rows in one DVE op:
                    # masked softmax = sum(mask*exp*V) / sum(mask*exp)
                    nc.vector.scalar_tensor_tensor(
                        out=vm, in0=ps, scalar=bv_s,
                        in1=mrow_s[:, bsl], op0=ADD, op1=MULT)
                    pvt = psProj.tile([P, nch, E], BF16, tag="proj",
                                      name=f"vt_{c0}")
                    for ci in range(nch):
                        nc.tensor.transpose(
                            pvt[:, ci], vm[:, ci * P:(ci + 1) * P],
                            ident_b[0:E, 0:E])
                    nc.vector.tensor_copy(out=V1[:, c0:c0 + nch, 0:E],
                                          in_=pvt)

                def block_scores(bi, qbs):
                    c0, nch = blocks[bi]
                    for qb in qbs:
                        for cp, ncp in _pairs(c0, nch):
                            scores_exp_pv(qb, cp, ncp)

                # block 0 first so exp starts as early as possible: run all
                # four block-0 score tiles back to back (PV emission held
                # until V1 block 0 exists); q blocks 2/3 project later
                # (their staging DMAs arrive mid-stream)
                qproj(qst.pop(0), 0)
                proj_dup(kst.pop(0), wk_b, bk_s, KT, *blocks[0])
                hold_pv[0] = True
                block_scores(0, [0])
                qproj(qst.pop(1), 1)
                block_scores(0, [1])
                vproj_block(0)
                hold_pv[0] = False
                while len(pending) > 1:
                    emit_pv(pending.pop(0))
                for bi in range(1, len(blocks)):
                    proj_dup(kst.pop(bi), wk_b, bk_s, KT, *blocks[bi])
                    vproj_block(bi)
                    block_scores(bi, stream_qbs)
                    if bi == 1:
                        qproj(qst.pop(2), 2)
                    elif bi == 2:
                        qproj(qst.pop(3), 3)

            # ---------------- stage 2: remaining query blocks ----------------
            with tc.tile_pool(name="psO", bufs=2, space="PSUM") as psO:
                flush_pv()
                pending_stream_epi = list(stream_qbs)
                # free pv0 so the first post query block can start accumulating
                epilogue(pending_stream_epi.pop(0), psO)

                def drain_stream_epi():
                    while pending_stream_epi:
                        epilogue(pending_stream_epi.pop(0), psO)

                for qb in post_qbs:
                    pv_tiles[qb] = psPV.tile([E + 1, TB], F32,
                                             tag=f"pv{qb % 2}", name=f"pv_{qb}")
                    first = True
                    for bi in range(len(blocks)):
                        block_scores(bi, [qb])
                        if not first:
                            drain_stream_epi()
                        first = False
                    drain_stream_epi()
                    flush_pv()
                    epilogue(qb, psO)


def build_nc(nt2):
    t2 = nt2 * P
    nc = bacc.Bacc()
    qh = nc.declare_dram_parameter("qh", [P, NQB, DC, TB], BF16, isOutput=False)
    kh = nc.declare_dram_parameter("kh", [P, nt2, DC, P], BF16, isOutput=False)
    vh = nc.declare_dram_parameter("vh", [P, nt2, DC, P], BF16, isOutput=False)
    mh = nc.declare_dram_parameter("mh", [P, nt2], F32, isOutput=False)
    mrow = nc.declare_dram_parameter("mrow", [t2], BF16, isOutput=False)
    Wq = nc.declare_dram_parameter("Wq", [P, DC, E], BF16, isOutput=False)
    bq2 = nc.declare_dram_parameter("bq2", [P], F32, isOutput=False)
    Wk = nc.declare_dram_parameter("Wk", [P, DC, E], BF16, isOutput=False)
    bk2 = nc.declare_dram_parameter("bk2", [P], F32, isOutput=False)
    Wv = nc.declare_dram_parameter("Wv", [P, DC, E], BF16, isOutput=False)
    bv = nc.declare_dram_parameter("bv", [E], F32, isOutput=False)
    out = nc.declare_dram_parameter("out", [P, T1L // P, E], F32,
                                    isOutput=True)
    with tile.TileContext(nc) as tc:
        build_body(tc, nc, nt2, qh[:], kh[:], vh[:], mh[:], mrow[:], Wq[:],
                   bq2[:], Wk[:], bk2[:], Wv[:], bv[:], out[:])
    nc.compile()
    return nc


_NC_CACHE = {}


def _get_nc(nt2):
    if nt2 not in _NC_CACHE:
        _NC_CACHE[nt2] = build_nc(nt2)
    return _NC_CACHE[nt2]


def make_in_maps(nt2, q, k, v, mask, Wq, bq, Wk, bk, Wv, bv):
    bf = ml_dtypes.bfloat16
    f32 = np.float32
    t2k = nt2 * P

    def qx(x):  # [t, 512] fp32 -> [128, NQB, 4, 512] bf16 p-major
        xt = np.asarray(x, f32).astype(bf).T
        xt = xt.reshape(DC, P, NQB, TB).transpose(1, 2, 0, 3)
        return np.ascontiguousarray(xt)

    def kx(x):  # [t2k, 512] fp32 -> [128, nt2, 4, 128] bf16 p-major
        xt = np.asarray(x, f32).astype(bf).T              # [512, t2k]
        xt = xt.reshape(DC, P, nt2, P).transpose(1, 2, 0, 3)
        return np.ascontiguousarray(xt)

    def wh(W):
        Wr = np.asarray(W, f32).astype(bf).reshape(DC, P, E).transpose(1, 0, 2)
        return np.ascontiguousarray(Wr)

    shared = {
        "Wq": wh(Wq), "Wk": wh(Wk), "Wv": wh(Wv),
        "bq2": np.concatenate([bq, bq]).astype(f32),
        "bk2": np.concatenate([bk, bk]).astype(f32),
        "bv": np.ascontiguousarray(bv, f32),
    }
    per_b = []
    for b in range(B):
        m = np.asarray(mask[b, 0], f32)
        # masked keys last; dropping trailing masked keys is exact
        order = np.argsort(1.0 - m, kind="stable")[:t2k]
        ms = m[order]
        per_b.append({
            "kh": kx(np.asarray(k[b], f32)[order]),
            "vh": kx(np.asarray(v[b], f32)[order]),
            "mh": np.ascontiguousarray(ms.reshape(nt2, P).T),
            "mrow": np.ascontiguousarray(ms.astype(bf)),
        })
    in_maps = []
    for c in range(8):
        b, h = divmod(c, 2)
        in_maps.append({
            "qh": qx(q[b, h * T1L:(h + 1) * T1L]),
            **per_b[b],
            **shared,
        })
    return in_maps


def assemble_out(results):
    out = np.empty((B, T1, E), np.float32)
    for c in range(8):
        b, h = divmod(c, 2)
        # device out is [128, 16, 64] p-major -> [2048, 64]
        o = results[c]["out"].transpose(1, 0, 2).reshape(T1L, E)
        out[b, h * T1L:(h + 1) * T1L] = o
    return out


def run(inputs, trace=False):
    from concourse.bass_utils import run_bass_kernel_spmd
    _install_ntff_hook()
    # the cut variant is exact when every batch keeps all unmasked keys
    unmasked = np.asarray(inputs["mask"], np.float32).reshape(B, T2)
    max_keep = int((unmasked != 0.0).sum(axis=1).max())
    nt2 = NT2_CUT if max_keep <= NT2_CUT * P else NT2_FULL
    nc = _get_nc(nt2)
    in_maps = make_in_maps(nt2, **inputs)
    res = run_bass_kernel_spmd(nc, in_maps, list(range(8)), trace=trace)
    return assemble_out(res.results), res


def kernel(q, k, v, mask, Wq, bq, Wk, bk, Wv, bv):
    out, _ = run(dict(q=q, k=k, v=v, mask=mask, Wq=Wq, bq=bq, Wk=Wk, bk=bk,
                      Wv=Wv, bv=bv))
    return out


# revision 21
# speedup vs baseline: 1.2432x; 1.0055x over previous
"""Bass/Tile TRN2 kernel for nn_AttentionHead: single-head attention with
q/k/v projections (512->64), key mask, softmax over 4096 keys.

Sharding: 8 cores; core c handles batch c//2, query-half c%2 (2048 queries),
with that batch's full k/v replicated. No collectives.

v6: on top of v5 (host bf16 cast + [d,t] transpose + p-major layouts, all
DMA on the two HWDGE rings, column-packed dup projections, DVE mask fold,
PE kept exclusively for matmul work and ACT for exp):
  - keys are sorted per batch so masked keys come last, and a fixed count
    of trailing masked keys is dropped (they contribute exactly zero to
    both the softmax numerator and denominator, so this is exact). The
    kernel compiles a 29-chunk (3712-key) variant used when every batch
    has >= 384 masked keys, and falls back to the full 32-chunk variant
    otherwise.
  - k/v stream DMAs issued per block in consumption order (early blocks
    individually, later ones batched) so the PE never waits on a batch
    that also carries a later block.

Per-core dataflow:
  - HWDGE loads qT/kT/vT bf16 [128, nch, 4, 128] tiles (d on partitions)
  - QT/KT [128, t] = W^T x^T duplicated on partitions 64-127 via PE column
    packing (tile_position (0,0)/(0,64)) in one moving pass, one DVE evac
  - V: V^T proj psum -> one DVE scalar_tensor_tensor (+bias, *mask) ->
    bf16 -> PE transpose -> psum -> one DVE copy -> V1 [t2, 65] whose
    col 64 holds the mask (masked softmax == sum(mask*exp*V)/sum(mask*exp))
  - scores: S^T tiles [t2=128, 1024] = KT_chunk.T @ QT (contract e=64);
    chunk pairs run concurrently in array row groups 0-63/64-127
  - ScalarE: exp(0.125 * S^T), one call per [128, 1024] psum pair
  - PV: O^T[65, 512] += V1_chunk.T @ expS (row 64 = denominator); PV
    matmuls are emitted one tile behind the scores so the PE never stalls
  - epilogue: PE transpose [65,128] blocks, reciprocal + scale on VectorE
  - query blocks 0,1 stream with the k/v blocks; 2,3 run after from
    SBUF-resident KT/V1 (PSUM bank budget: 4 scores + 2 PV + 2 proj)
"""

import sys
import types

import numpy as np
import ml_dtypes

import concourse.bass as bass
import concourse.tile as tile
from concourse import bacc, mybir
from concourse.masks import make_identity

B, T1, T2, D, E = 4, 4096, 4096, 512, 64
P = 128
T1L = T1 // 2          # queries per core
DC = D // P            # 4 d-chunks
TB = 512               # q block size / nominal k-v block size
NQB = T1L // TB        # 4 query blocks
NT2_FULL = T2 // P     # 32 key chunks
NT2_CUT = 29           # 3712 keys: used when every batch has >=384 masked
F32 = mybir.dt.float32
BF16 = mybir.dt.bfloat16
EXPF = mybir.ActivationFunctionType.Exp
ADD = mybir.AluOpType.add
MULT = mybir.AluOpType.mult


def _install_ntff_hook():
    """Make trace=True usable under axon when antenv.axon_hooks is absent."""
    try:
        import antenv.axon_hooks  # noqa: F401
        return
    except ImportError:
        pass
    try:
        from trn_agent_boot.trn_boot import _ntff_profile_via_ctypes
        hook = _ntff_profile_via_ctypes("/opt/axon/libaxon_pjrt.so")
    except Exception:
        hook = None
    mod = types.ModuleType("antenv.axon_hooks")
    mod.get_axon_ntff_profile_hook = lambda: hook
    mod.set_axon_ntff_profile_hook = lambda h: None
    sys.modules["antenv.axon_hooks"] = mod


def _blocks(nt2):
    """Key chunk blocks [(c0, nch)], 4-chunk blocks with a ragged tail."""
    return [(c0, min(4, nt2 - c0)) for c0 in range(0, nt2, 4)]


def _pairs(c0, nch):
    """Score-tile chunk groups within a block (pairs, ragged tail)."""
    return [(c, min(2, c0 + nch - c)) for c in range(c0, c0 + nch, 2)]


def build_body(tc, nc, nt2, qh, kh, vh, mh, mrow, Wq, bq2, Wk, bk2, Wv, bv,
               out):
    t2 = nt2 * P
    blocks = _blocks(nt2)
    with (
        tc.tile_pool(name="consts", bufs=1) as consts,
        tc.tile_pool(name="persist", bufs=1) as persist,
        tc.tile_pool(name="qstage", bufs=NQB) as qstage,
        tc.tile_pool(name="kstage", bufs=1) as kstage,
        tc.tile_pool(name="vstage", bufs=1) as vstage,
    ):
        # ---- prologue DMAs: first blocks on sync, consts on scalar ----
        qst, kst, vst = {}, {}, {}

        def stage_q(tb, eng):
            qst[tb] = qstage.tile([P, DC, TB], BF16, tag="st",
                                  name=f"st_q_{tb}")
            eng.dma_start(out=qst[tb], in_=qh[:, tb])

        def kv_dma(dst, pool, src, nm, bi):
            # tag-rotated staging (bufs=3) self-throttles the k/v stream to
            # consumption pace: later blocks transfer just-in-time instead
            # of fighting the early blocks for DMA/SBUF bandwidth
            c0, nch = blocks[bi]
            tl = pool.tile([P, nch, DC, P], BF16, tag="st",
                           name=f"st_{nm}_{c0}", bufs=4)
            nc.sync.dma_start(out=tl, in_=src[:, c0:c0 + nch])
            dst[bi] = tl

        stage_q(0, nc.sync)
        kv_dma(kst, kstage, kh, "k", 0)
        kv_dma(kst, kstage, kh, "k", 1)
        kv_dma(vst, vstage, vh, "v", 0)
        kv_dma(vst, vstage, vh, "v", 1)

        bq_s = consts.tile([P, 1], F32)
        nc.scalar.dma_start(out=bq_s, in_=bq2[:, None])
        wq_b = consts.tile([P, DC, E], BF16)
        nc.scalar.dma_start(out=wq_b, in_=Wq)
        wk_b = consts.tile([P, DC, E], BF16)
        nc.scalar.dma_start(out=wk_b, in_=Wk)
        wv_b = consts.tile([P, DC, E], BF16)
        nc.scalar.dma_start(out=wv_b, in_=Wv)
        bk_s = consts.tile([P, 1], F32)
        nc.scalar.dma_start(out=bk_s, in_=bk2[:, None])
        bv_s = consts.tile([E, 1], F32)
        nc.scalar.dma_start(out=bv_s, in_=bv[:, None])
        stage_q(1, nc.scalar)
        # per-key mask values, bf16, replicated across 64 partitions
        # (stride-0 partition DMA) for the V mask fold
        mrow_s = consts.tile([E, t2], BF16)
        nc.scalar.dma_start(
            out=mrow_s,
            in_=bass.AP(tensor=mrow.tensor, offset=mrow.offset,
                        ap=[[0, E], mrow.ap[0]]))
        # mask values per key, [partition = t2 % 128, col = t2 // 128]
        mk = consts.tile([P, nt2], F32)
        nc.scalar.dma_start(out=mk, in_=mh)

        # remaining k/v stream on sync, throttled by staging rotation;
        # q blocks 2-3 (needed only in stage 2) ride between early blocks
        nb = len(blocks)
        for bi in range(2, nb):
            kv_dma(kst, kstage, kh, "k", bi)
            kv_dma(vst, vstage, vh, "v", bi)
            if bi == 2:
                stage_q(2, nc.sync)
                stage_q(3, nc.sync)

        # warmup exp: loads the ACT table set now that the scalar ring is
        # done issuing prologue DMAs
        warm = consts.tile([P, 1], BF16)
        nc.scalar.activation(out=warm, in_=bq_s, func=EXPF, scale=1.0)

        ident_b = consts.tile([P, P], BF16)
        make_identity(nc, ident_b)
        ident_f = consts.tile([P, P], F32)
        make_identity(nc, ident_f)

        QT = persist.tile([P, T1L], BF16)
        KT = persist.tile([P, t2], BF16)
        V1 = persist.tile([P, nt2, E + 1], BF16)
        out_sb = persist.tile([P, T1L // P, E], F32)

        # the "ones" column of V1 carries the mask directly: the masked
        # softmax denominator is sum(mask * exp)
        nc.vector.tensor_copy(out=V1[:, :, E], in_=mk)

        pv_tiles = {}
        pending = []

        with (
            tc.tile_pool(name="expp", bufs=4) as expp,
            tc.tile_pool(name="ep", bufs=3) as ep,
            tc.tile_pool(name="psS", bufs=2, space="PSUM") as psS,
            tc.tile_pool(name="psPV", bufs=1, space="PSUM") as psPV,
        ):
            def emit_pv(item):
                qb, c0, ncp, ex = item
                for u in range(ncp):
                    c = c0 + u
                    nc.tensor.matmul(
                        pv_tiles[qb], V1[:, c, :], ex[:, u * TB:(u + 1) * TB],
                        start=(c == 0), stop=(c == nt2 - 1))

            hold_pv = [False]

            def scores_exp_pv(qb, c0, ncp):
                q0 = qb * TB
                w = ncp * TB
                ps = psS.tile([P, w], F32, tag="s", name=f"s_{qb}_{c0}")
                for u in range(ncp):
                    c = c0 + u
                    rg = E * (u % 2)
                    nc.tensor.matmul(
                        ps[:, u * TB:(u + 1) * TB],
                        KT[rg:rg + E, c * P:(c + 1) * P],
                        QT[rg:rg + E, q0:q0 + TB], start=True, stop=True,
                        tile_position=(rg, 0))
                ex = expp.tile([P, w], BF16, tag="e", name=f"e_{qb}_{c0}")
                nc.scalar.activation(out=ex, in_=ps, func=EXPF, scale=0.125)
                pending.append((qb, c0, ncp, ex))
                while not hold_pv[0] and len(pending) > 1:
                    emit_pv(pending.pop(0))

            def flush_pv():
                while pending:
                    emit_pv(pending.pop(0))

            def epilogue(qb, psO):
                pvt = pv_tiles.pop(qb)
                q0 = qb * TB
                ov = ep.tile([E + 1, TB], F32, tag="ov", name=f"ov_{qb}")
                nc.vector.tensor_copy(out=ov, in_=pvt)
                for j in range(TB // P):
                    po = psO.tile([P, E + 1], F32, tag="o", name=f"o_{qb}_{j}")
                    nc.tensor.transpose(
                        po, ov[:, j * P:(j + 1) * P],
                        ident_f[0:E + 1, 0:E + 1])
                    rec = ep.tile([P, 1], F32, tag="rec", name=f"rec_{qb}_{j}")
                    nc.vector.reciprocal(rec, po[:, E:E + 1])
                    n = (q0 + j * P) // P
                    nc.vector.tensor_scalar_mul(
                        out_sb[:, n, :], po[:, 0:E], rec)
                    # per-chunk output DMA keeps the final-epilogue tail short
                    nc.sync.dma_start(out=out[:, n:n + 1, :],
                                      in_=out_sb[:, n:n + 1, :])

            # ---------------- stage 1: stream ----------------
            stream_qbs = [0, 1]
            post_qbs = [2, 3]

            with tc.tile_pool(name="psProj", bufs=2, space="PSUM") as psProj:
                for qb in stream_qbs:
                    pv_tiles[qb] = psPV.tile([E + 1, TB], F32,
                                             tag=f"pv{qb % 2}", name=f"pv_{qb}")

                def proj_dup(st, w_b, b_s, dst, c0, nch):
                    # projection written to partitions 0:64 and duplicated to
                    # 64:128 in one moving pass via PE column packing
                    w = nch * P
                    ps = psProj.tile([P, w], F32, tag="proj",
                                     name=f"pp_{dst.tensor.name}_{c0}")
                    for j in range(DC):
                        nc.tensor.matmul(
                            ps[0:E, :], w_b[:, j], st[:, :, j, :],
                            start=(j == 0), stop=(j == DC - 1),
                            tile_position=(0, 0))
                        nc.tensor.matmul(
                            ps[E:P, :], w_b[:, j], st[:, :, j, :],
                            start=(j == 0), stop=(j == DC - 1),
                            tile_position=(0, E))
                    nc.vector.tensor_scalar_add(
                        dst[:, c0 * P:c0 * P + w], ps, b_s)

                def qproj(st, tb):
                    # q staging keeps the [P, DC, TB] layout
                    ps = psProj.tile([P, TB], F32, tag="proj",
                                     name=f"pp_q_{tb}")
                    for j in range(DC):
                        nc.tensor.matmul(
                            ps[0:E, :], wq_b[:, j], st[:, j],
                            start=(j == 0), stop=(j == DC - 1),
                            tile_position=(0, 0))
                        nc.tensor.matmul(
                            ps[E:P, :], wq_b[:, j], st[:, j],
                            start=(j == 0), stop=(j == DC - 1),
                            tile_position=(0, E))
                    nc.vector.tensor_scalar_add(
                        QT[:, tb * TB:(tb + 1) * TB], ps, bq_s)

                def vproj_block(bi):
                    c0, nch = blocks[bi]
                    st = vst.pop(bi)
                    w = nch * P
                    ps = psProj.tile([E, w], F32, tag="proj",
                                     name=f"psv_{c0}")
                    for j in range(DC):
                        nc.tensor.matmul(
                            ps, wv_b[:, j], st[:, :, j, :],
                            start=(j == 0), stop=(j == DC - 1))
                    bsl = slice(c0 * P, c0 * P + w)
                    vm = ep.tile([E, w], BF16, tag="vm", name=f"vm_{c0}")
                    # fold bias and key mask into V rows in one DVE op:
                    # masked softmax = sum(mask*exp*V) / sum(mask*exp)
                    nc.vector.scalar_tensor_tensor(
                        out=vm, in0=ps, scalar=bv_s,
                        in1=mrow_s[:, bsl], op0=ADD, op1=MULT)
                    pvt = psProj.tile([P, nch, E], BF16, tag="proj",
                                      name=f"vt_{c0}")
                    for ci in range(nch):
                        nc.tensor.transpose(
                            pvt[:, ci], vm[:, ci * P:(ci + 1) * P],
                            ident_b[0:E, 0:E])
                    nc.vector.tensor_copy(out=V1[:, c0:c0 + nch, 0:E],
                                          in_=pvt)

                def block_scores(bi, qbs):
                    c0, nch = blocks[bi]
                    for qb in qbs:
                        for cp, ncp in _pairs(c0, nch):
                            scores_exp_pv(qb, cp, ncp)

                # block 0 first so exp starts as early as possible: run all
                # four block-0 score tiles back to back (PV emission held
                # until V1 block 0 exists); q blocks 2/3 project later
                # (their staging DMAs arrive mid-stream)
                qproj(qst.pop(0), 0)
                proj_dup(kst.pop(0), wk_b, bk_s, KT, *blocks[0])
                hold_pv[0] = True
                block_scores(0, [0])
                qproj(qst.pop(1), 1)
                block_scores(0, [1])
                vproj_block(0)
                hold_pv[0] = False
                while len(pending) > 1:
                    emit_pv(pending.pop(0))
                for bi in range(1, len(blocks)):
                    proj_dup(kst.pop(bi), wk_b, bk_s, KT, *blocks[bi])
                    vproj_block(bi)
                    block_scores(bi, stream_qbs)
                    if bi == 1:
                        qproj(qst.pop(2), 2)
                    elif bi == 2:
                        qproj(qst.pop(3), 3)

            # ---------------- stage 2: remaining query blocks ----------------
            with tc.tile_pool(name="psO", bufs=2, space="PSUM") as psO:
                flush_pv()
                pending_stream_epi = list(stream_qbs)
                # free pv0 so the first post query block can start accumulating
                epilogue(pending_stream_epi.pop(0), psO)

                def drain_stream_epi():
                    while pending_stream_epi:
                        epilogue(pending_stream_epi.pop(0), psO)

                for qb in post_qbs:
                    pv_tiles[qb] = psPV.tile([E + 1, TB], F32,
                                             tag=f"pv{qb % 2}", name=f"pv_{qb}")
                    first = True
                    for bi in range(len(blocks)):
                        block_scores(bi, [qb])
                        if not first:
                            drain_stream_epi()
                        first = False
                    drain_stream_epi()
                    flush_pv()
                    epilogue(qb, psO)


def build_nc(nt2):
    t2 = nt2 * P
    nc = bacc.Bacc()
    qh = nc.declare_dram_parameter("qh", [P, NQB, DC, TB], BF16, isOutput=False)
    kh = nc.declare_dram_parameter("kh", [P, nt2, DC, P], BF16, isOutput=False)
    vh = nc.declare_dram_parameter("vh", [P, nt2, DC, P], BF16, isOutput=False)
    mh = nc.declare_dram_parameter("mh", [P, nt2], F32, isOutput=False)
    mrow = nc.declare_dram_parameter("mrow", [t2], BF16, isOutput=False)
    Wq = nc.declare_dram_parameter("Wq", [P, DC, E], BF16, isOutput=False)
    bq2 = nc.declare_dram_parameter("bq2", [P], F32, isOutput=False)
    Wk = nc.declare_dram_parameter("Wk", [P, DC, E], BF16, isOutput=False)
    bk2 = nc.declare_dram_parameter("bk2", [P], F32, isOutput=False)
    Wv = nc.declare_dram_parameter("Wv", [P, DC, E], BF16, isOutput=False)
    bv = nc.declare_dram_parameter("bv", [E], F32, isOutput=False)
    out = nc.declare_dram_parameter("out", [P, T1L // P, E], F32,
                                    isOutput=True)
    with tile.TileContext(nc) as tc:
        build_body(tc, nc, nt2, qh[:], kh[:], vh[:], mh[:], mrow[:], Wq[:],
                   bq2[:], Wk[:], bk2[:], Wv[:], bv[:], out[:])
    nc.compile()
    return nc


_NC_CACHE = {}


def _get_nc(nt2):
    if nt2 not in _NC_CACHE:
        _NC_CACHE[nt2] = build_nc(nt2)
    return _NC_CACHE[nt2]


def make_in_maps(nt2, q, k, v, mask, Wq, bq, Wk, bk, Wv, bv):
    bf = ml_dtypes.bfloat16
    f32 = np.float32
    t2k = nt2 * P

    def qx(x):  # [t, 512] fp32 -> [128, NQB, 4, 512] bf16 p-major
        xt = np.asarray(x, f32).astype(bf).T
        xt = xt.reshape(DC, P, NQB, TB).transpose(1, 2, 0, 3)
        return np.ascontiguousarray(xt)

    def kx(x):  # [t2k, 512] fp32 -> [128, nt2, 4, 128] bf16 p-major
        xt = np.asarray(x, f32).astype(bf).T              # [512, t2k]
        xt = xt.reshape(DC, P, nt2, P).transpose(1, 2, 0, 3)
        return np.ascontiguousarray(xt)

    def wh(W):
        Wr = np.asarray(W, f32).astype(bf).reshape(DC, P, E).transpose(1, 0, 2)
        return np.ascontiguousarray(Wr)

    shared = {
        "Wq": wh(Wq), "Wk": wh(Wk), "Wv": wh(Wv),
        "bq2": np.concatenate([bq, bq]).astype(f32),
        "bk2": np.concatenate([bk, bk]).astype(f32),
        "bv": np.ascontiguousarray(bv, f32),
    }
    per_b = []
    for b in range(B):
        m = np.asarray(mask[b, 0], f32)
        # masked keys last; dropping trailing masked keys is exact
        order = np.argsort(1.0 - m, kind="stable")[:t2k]
        ms = m[order]
        per_b.append({
            "kh": kx(np.asarray(k[b], f32)[order]),
            "vh": kx(np.asarray(v[b], f32)[order]),
            "mh": np.ascontiguousarray(ms.reshape(nt2, P).T),
            "mrow": np.ascontiguousarray(ms.astype(bf)),
        })
    in_maps = []
    for c in range(8):
        b, h = divmod(c, 2)
        in_maps.append({
            "qh": qx(q[b, h * T1L:(h + 1) * T1L]),
            **per_b[b],
            **shared,
        })
    return in_maps


def assemble_out(results):
    out = np.empty((B, T1, E), np.float32)
    for c in range(8):
        b, h = divmod(c, 2)
        # device out is [128, 16, 64] p-major -> [2048, 64]
        o = results[c]["out"].transpose(1, 0, 2).reshape(T1L, E)
        out[b, h * T1L:(h + 1) * T1L] = o
    return out


def run(inputs, trace=False):
    from concourse.bass_utils import run_bass_kernel_spmd
    _install_ntff_hook()
    # the cut variant is exact when every batch keeps all unmasked keys
    unmasked = np.asarray(inputs["mask"], np.float32).reshape(B, T2)
    max_keep = int((unmasked != 0.0).sum(axis=1).max())
    nt2 = NT2_CUT if max_keep <= NT2_CUT * P else NT2_FULL
    nc = _get_nc(nt2)
    in_maps = make_in_maps(nt2, **inputs)
    res = run_bass_kernel_spmd(nc, in_maps, list(range(8)), trace=trace)
    return assemble_out(res.results), res


def kernel(q, k, v, mask, Wq, bq, Wk, bk, Wv, bv):
    out, _ = run(dict(q=q, k=k, v=v, mask=mask, Wq=Wq, bq=bq, Wk=Wk, bk=bk,
                      Wv=Wv, bv=bv))
    return out
